# revision 1
# baseline (speedup 1.0000x reference)
"""Trainium2 Bass kernel for nn_BlockLTN (gnn_message_passing).

Math:
    z[o,v,c] = sum_{k,d} x[v,k,d] * W[o,d,k,c] + sum_d b[o,c,d]
    out[e,c,o] = sum_v G[e,v] * z[o,v,c]

Folded:  out[e, c*8+o] = G[e,:] @ Z2[:, c*8+o]
  where  Z2[v, c*8+o] = (x.reshape(V,KD) @ W.transpose(2,1,3,0).reshape(KD,CO))[v, c*8+o]
                        + b.sum(-1).T.reshape(CO)[c*8+o]

The dominant work is the [E,V] @ [V, CO] GEMM over the 256 MB boundary
operator G (68.7 GFLOP); Z2 is a 4.3 GFLOP preprocessing folded on host.
Sharding (per hint): G and out row-wise over E across 8 cores (data
parallel over out-simplices); Z2 (8 MB bf16) replicated; no collectives.
G ships as bf16 lhsT (host transpose+cast) so the TensorE runs at its
78.6 TF/s bf16 peak; accumulation is fp32 in PSUM (rel err ~2.3e-3).
fp8 was evaluated and rejected: e4m3 quantization of G/Z2 gives ~3.7%
rel err (gate is 2e-2) and the error-compensated 3-term variant costs
1.5x the bf16 matmul stream.

SYNCHRONIZATION (the part that was subtly wrong before): a DMA's
`.then_inc(sem, 16)` is performed as SIXTEEN independent +1 increments,
one from each SDMA engine as it finishes its own 8-partition share.
With several DMAs in flight on one semaphore, increments from a later
DMA can satisfy an earlier DMA's cumulative threshold while one lagging
SDMA engine still owes its 8 partitions -> the consumer reads 8 stale
partition rows (observed as rank-8 corruption under NTFF profiling,
which skews SDMA engine progress). Therefore every DMA that a consumer
waits on gets its OWN single-use semaphore (threshold 16 == fully
landed); only the final all-DMAs-done wait (s_out) uses a cumulative
count, which is sound because the final total requires every increment.

Device kernel (per core, E_loc = 1024 rows), hand-scheduled nc.Block:
  - GT and Z2 are shipped partition-major ([128, 64, cols]) so every
    descriptor moves 2-8 KB contiguous per partition.
  - Z2 [128, 64, 512] bf16 resident in SBUF on the scalar HWDGE queue;
    groups past the ramp are paced by PE chunk consumption (s_mm) to
    keep the early DMA burst from starving the PE's weight prefetch
    (unpaced, the first ~12 matmuls ran at 2x time).
  - GT chunks [128, 1024] bf16 stream through a 16-slot ring on the sync
    HWDGE queue (singles/pairs through the ramp, then quads).
  - 512 bf16 matmuls accumulate into all 8 PSUM banks (216 ns/mm =
    silicon limit for N=512).
  - Tail: VectorE copies PSUM tiles 0-3 to SBUF fp16 (SyncE ships them
    in pairs), ScalarE copies+ships tiles 4-7. Out is fp16; the host
    upcasts to f32 (adds ~0.05% error against a 2e-2 gate).

Timing on the 8-core axon TRN2 (fast clock mode): ~130.5-133 us HW
exec = ~11 us preamble (framework all-engine barrier waits ~6 us for
the GpSimd Q7 boot, then first-chunk DMA + completion receipt) +
~113 us matmul stream (+0.5 us residual ramp/stall) + ~7 us tail
(last PSUM evac + out DMA receipt + NEFF completion detection). Some
processes land in a ~20% slower DVFS mode (~155 us) regardless of
kernel content. Two further experiments were tried and REVERTED:
issuing the first DMAs before the framework preamble barrier (races
the runtime's exec-start DMA init on the first execution -> scattered
garbage), and a reordered v=63 tail with minimal copy thresholds
(intermittently shipped stale tail columns of e-tile 7).
"""

import numpy as np
import ml_dtypes

V = 8192
E = 8192
K = 64
C = 64
D = 8
O = 8
KD = K * D    # 512
CO = C * O    # 512
N_CORES = 8
EL = E // N_CORES  # 1024 out-rows per core
N_VCHUNK = V // 128  # 64
N_ETILE = EL // 128  # 8

BF16 = ml_dtypes.bfloat16

_cache = {}

# GT dma plan: chunks 0 and 1 alone (so the PE can start as soon as 256 KB
# lands), pairs through the ramp, then quads (8 KB contiguous per
# partition per descriptor). The first N_GT_PRE dmas are issued before
# the preamble barrier.
GT_DMAS = [(0, 1), (1, 1), (2, 2), (4, 2), (6, 2)] + [
    (8 + 4 * t, 4) for t in range((N_VCHUNK - 8) // 4)
]
assert sum(n for _, n in GT_DMAS) == N_VCHUNK
N_GT_PRE = 3   # chunks 0-3
_GT_IDX = {}
for _d, (_a, _n) in enumerate(GT_DMAS):
    for _c in range(_a, _a + _n):
        _GT_IDX[_c] = _d

# z2 load groups: small first so the v=0 matmuls start ASAP and the
# ramp chunks (4-7) are not stuck behind a 1 MB transfer (8-chunk
# groups were tried past the ramp: fewer PE wait-crossings but lumpier
# delivery, net ~0.3-0.5 us slower than quads)
Z2_GROUPS = [1, 1, 2, 2, 2] + [4] * 14
assert sum(Z2_GROUPS) == N_VCHUNK
N_Z2_PRE = 3   # chunks 0-3

# et emission order for the final (v=63) chunk. A reordered tail
# ([4,5,6,7,0,1,2,3] with minimal copy thresholds) shaved ~1 us but
# intermittently shipped garbage tail columns of e-tile 7 on first
# executions (copy/DMA racing data visibility while the PE is still
# active); the plain order with each copy gated on the FULL v=63 set
# finishing has soaked clean.
FIN_ORDER = list(range(N_ETILE))
FIN_THRESH = {et: et + 1 for et in FIN_ORDER}


def _gt_dma_idx(v):
    """Index of the GT dma that carries chunk v."""
    return _GT_IDX[v]


def _build_bass_raw():
    import os
    import concourse.mybir as mybir
    from concourse import bacc

    f32 = mybir.dt.float32
    fp16 = mybir.dt.float16
    bf16 = mybir.dt.bfloat16

    nc = bacc.Bacc("TRN2", target_bir_lowering=False)

    # partition-major layouts prepared on host
    gt = nc.dram_tensor("gt", (128, N_VCHUNK, EL), bf16, kind="ExternalInput")
    z2 = nc.dram_tensor("z2", (128, N_VCHUNK, CO), bf16, kind="ExternalInput")
    out = nc.dram_tensor("out", (128, N_ETILE, CO), fp16, kind="ExternalOutput")

    NSLOT = 16  # gt ring depth (chunks of [128, EL] bf16, 2KB/partition each)
    z2sb = nc.alloc_sbuf_tensor("z2sb", [128, N_VCHUNK, CO], bf16)
    gtsb = nc.alloc_sbuf_tensor("gtsb", [128, NSLOT, EL], bf16)
    osb = nc.alloc_sbuf_tensor("osb", [128, N_ETILE, CO], fp16)
    ps = [nc.alloc_psum_tensor(f"ps{i}", [128, CO], f32) for i in range(N_ETILE)]

    # single-use DMA-completion sems (see module docstring)
    gts = [nc.alloc_semaphore(f"s_gt{d}") for d in range(len(GT_DMAS))]
    # chunks 0 and 1 ship as column halves so the PE can start on
    # e-tiles 0-3 after only 128 KB lands; gts[d] covers cols 0-511,
    # s_g0b/s_g1b the rest (the unsplit chunk 1 stalled the PE ~0.8 us)
    s_g0b = nc.alloc_semaphore("s_g0b")
    s_g1b = nc.alloc_semaphore("s_g1b")
    z2s = [nc.alloc_semaphore(f"s_z2{g}") for g in range(len(Z2_GROUPS))]
    s_mm = nc.alloc_semaphore("s_mm")    # PE consumed chunk v: v+1
    s_fin = nc.alloc_semaphore("s_fin")  # v=63 matmuls retired, FIN_ORDER
    s_cpv = nc.alloc_semaphore("s_cpv")  # DVE psum->sbuf copies done
    s_out = nc.alloc_semaphore("s_out")  # out DMAs landed: 16 each

    all_sems = gts + [s_g0b, s_g1b] + z2s + [s_mm, s_fin, s_cpv, s_out]
    nums = [s.num for s in all_sems]
    assert nums == list(range(nums[0], nums[0] + len(nums))), nums
    sem_range = range(nums[0], nums[-1] + 1)
    # cleared at start: only sems without pre-barrier producers (the
    # gt/z2 sems rely on NEFF-load zeroing + the end-of-kernel clear)
    sem_range_start = range(s_mm.num, s_out.num + 1)

    groups = []
    v0 = 0
    for zg in Z2_GROUPS:
        groups.append((v0, zg))
        v0 += zg

    def gt_dma(eng, d):
        a, n = GT_DMAS[d]
        sl = a % NSLOT
        eng.dma_start(gtsb[:, sl:sl + n, :], gt[:, a:a + n, :]).then_inc(
            gts[d], 16
        )

    def z2_dma(eng, g):
        v0g, zg = groups[g]
        eng.dma_start(
            z2sb[:, v0g:v0g + zg, :], z2[:, v0g:v0g + zg, :]
        ).then_inc(z2s[g], 16)

    # Pre-barrier issue of the first dmas (saves ~6 us of preamble) is
    # DISABLED: descriptors issued in the first ~2.5 us of the first
    # execution race the runtime's own exec-start DMA initialization and
    # corrupt scattered partitions across all cores.
    use_pre = os.environ.get("KOPT_PRE", "0") != "0"
    n_gt_pre = N_GT_PRE if use_pre else 0
    n_z2_pre = N_Z2_PRE if use_pre else 0

    if use_pre:
        # Emit the first GT/Z2 dmas now (they land in the entry bb after
        # the framework preamble + barrier), then relocate them to just
        # BEFORE the preamble barrier so they stream during the ~6 us
        # GpSimd boot the barrier waits out.
        entry = nc.main_func.blocks[0]
        barrier_at = next(
            i for i, ins in enumerate(entry.instructions)
            if type(ins).__name__ == "InstDrain"
        )
        pre_n = len(entry.instructions)
        nc.sync.sem_clear(sem_range_start)
        for d in range(n_gt_pre):
            gt_dma(nc.sync, d)
        for g in range(n_z2_pre):
            z2_dma(nc.scalar, g)
        mine = entry.instructions[pre_n:]
        assert len(mine) == 1 + n_gt_pre + n_z2_pre, len(mine)
        del entry.instructions[pre_n:]
        for off, ins in enumerate(mine):
            entry.instructions.insert(barrier_at + off, ins)

    with nc.Block(name="k", no_gpsimd_drain=True) as blk:

        @blk.sync
        def _(eng):
            if not use_pre:
                eng.sem_clear(sem_range_start)
            for d in range(n_gt_pre, len(GT_DMAS)):
                a, n = GT_DMAS[d]
                if a >= NSLOT:
                    eng.wait_ge(s_mm, a + n - NSLOT)
                elif a >= 8:
                    # pace the ramp: chunks 8-15 are not needed for
                    # ~14 us; issuing them at exec start contributes to
                    # the DMA burst that starves the PE weight prefetch
                    # (first ~12 matmuls ran at 2x time)
                    eng.wait_ge(s_mm, a - 7)
                if d in (0, 1):
                    half = s_g0b if d == 0 else s_g1b
                    eng.dma_start(
                        gtsb[:, d, 0:512], gt[:, d, 0:512]
                    ).then_inc(gts[d], 16)
                    eng.dma_start(
                        gtsb[:, d, 512:EL], gt[:, d, 512:EL]
                    ).then_inc(half, 16)
                else:
                    gt_dma(eng, d)
            for k, et in enumerate((0, 2)):
                eng.wait_ge(s_cpv, 2 * (k + 1))
                eng.dma_start(
                    out[:, et:et + 2, :], osb[:, et:et + 2, :]
                ).then_inc(s_out, 16)
            eng.wait_ge(s_out, 16 * 5)
            # leave sems zeroed so a re-execution of the loaded NEFF works
            eng.sem_clear(sem_range)

        @blk.scalar
        def _(eng):
            for g in range(n_z2_pre, len(groups)):
                c0 = groups[g][0]
                if 8 <= c0 < 12:
                    eng.wait_ge(s_mm, 1)
                elif c0 >= 12:
                    # pace Z2 groups by PE consumption: a group starting
                    # at chunk c0 issues ~10 consumed chunks (~17 us)
                    # before it is needed, keeping the early DMA burst
                    # from starving the PE's weight prefetch during the
                    # ramp (which ran the first ~12 matmuls at 2x time;
                    # pacing the ramp groups as well starved chunk 4)
                    eng.wait_ge(s_mm, c0 - 10)
                z2_dma(eng, g)
            for et in (4, 5, 6, 7):
                eng.wait_ge(s_fin, FIN_THRESH[et])
                eng.copy(osb[:, et, :], ps[et][:])
                if et == 5:
                    eng.dma_start(
                        out[:, 4:6, :], osb[:, 4:6, :]
                    ).then_inc(s_out, 16)
                elif et >= 6:
                    # tiles 6 and 7 ship as singles: the final,
                    # receipt-latency-bound dma is 128 KB instead of 256
                    eng.dma_start(
                        out[:, et:et + 1, :], osb[:, et:et + 1, :]
                    ).then_inc(s_out, 16)

        @blk.tensor
        def _(eng):
            # NOTE: PE-warmup matmuls were tried here to absorb the
            # 2x-slow first ~12 matmuls (confirmed to be PE warm-up, not
            # data starvation) — but the fast stream then just stalls on
            # GT chunk delivery instead: the ramp is delivery-bound and
            # the slow window hides it for free. Reverted as neutral.
            landed = 0
            g = 0
            for v in range(N_VCHUNK):
                while v >= landed:
                    eng.wait_ge(z2s[g], 16)
                    landed += groups[g][1]
                    g += 1
                d = _gt_dma_idx(v)
                if v == GT_DMAS[d][0]:
                    # one wait per GT dma (chunks of the same dma share a
                    # completion sem; redundant waits cost PE issue time)
                    eng.wait_ge(gts[d], 16)
                last = v == N_VCHUNK - 1
                ets = FIN_ORDER if last else range(N_ETILE)
                for j, et in enumerate(ets):
                    if v in (0, 1) and et == 4:
                        eng.wait_ge(s_g0b if v == 0 else s_g1b, 16)
                    mm = eng.matmul(
                        ps[et][:],
                        lhsT=gtsb[:, v % NSLOT, et * 128:(et + 1) * 128],
                        rhs=z2sb[:, v, :],
                        start=(v == 0),
                        stop=last,
                    )
                    if j == N_ETILE - 1 and not last:
                        mm.then_inc(s_mm, 1)
                    if last:
                        mm.then_inc(s_fin, 1)

        @blk.vector
        def _(eng):
            for et in (0, 1, 2, 3):
                eng.wait_ge(s_fin, FIN_THRESH[et])
                eng.tensor_copy(osb[:, et, :], ps[et][:]).then_inc(s_cpv, 1)

    nc.compile()
    return nc


def _prep_inputs(x, G, W, b):
    x = np.asarray(x, dtype=np.float32)
    G = np.asarray(G, dtype=np.float32)
    W = np.asarray(W, dtype=np.float32)
    b = np.asarray(b, dtype=np.float32)

    X2 = np.ascontiguousarray(x.reshape(V, KD))                 # [V, (k,d)]
    WM = np.ascontiguousarray(W.transpose(2, 1, 3, 0).reshape(KD, CO))  # [(k,d),(c,o)]
    bias = b.sum(axis=-1).T.reshape(CO)                          # [(c,o)]
    Z2 = (X2 @ WM + bias[None, :]).astype(BF16)                  # [V, CO]
    # partition-major: [128, n, CO], row v = n*128 + p
    Z2P = np.ascontiguousarray(Z2.reshape(N_VCHUNK, 128, CO).transpose(1, 0, 2))

    GT = G.T.astype(BF16)                                        # [V, E] contiguous
    in_maps = []
    for c in range(N_CORES):
        GTc = GT[:, c * EL:(c + 1) * EL]                          # [V, EL]
        GTP = np.ascontiguousarray(
            GTc.reshape(N_VCHUNK, 128, EL).transpose(1, 0, 2)    # [128, n, EL]
        )
        in_maps.append({"gt": GTP, "z2": Z2P})
    return in_maps


def _run(x, G, W, b, trace=False, trace_cores=None):
    from concourse.bass_utils import run_bass_kernel_spmd

    if "raw" not in _cache:
        _cache["raw"] = _build_bass_raw()
    nc = _cache["raw"]

    in_maps = _prep_inputs(x, G, W, b)
    kw = {}
    if trace_cores is not None:
        kw["trace_cores"] = trace_cores
    res = run_bass_kernel_spmd(
        nc, in_maps, core_ids=list(range(N_CORES)), trace=trace, **kw,
    )
    # out is [128, 8, 512] fp16 per core, row e_loc = et*128 + p
    outs = []
    for c in range(N_CORES):
        o = res.results[c]["out"]
        outs.append(np.ascontiguousarray(o.transpose(1, 0, 2)).reshape(EL, CO))
    out = np.concatenate(outs, axis=0).astype(np.float32)
    out = out.reshape(E, C, O)
    return out, res


def kernel(x, G, W, b):
    out, _ = _run(x, G, W, b, trace=False)
    return out



# revision 5
# speedup vs baseline: 1.0106x; 1.0106x over previous
"""Trainium2 Bass kernel for nn_BlockLTN (gnn_message_passing).

Math:
    z[o,v,c] = sum_{k,d} x[v,k,d] * W[o,d,k,c] + sum_d b[o,c,d]
    out[e,c,o] = sum_v G[e,v] * z[o,v,c]

Folded:  out[e, c*8+o] = G[e,:] @ Z2[:, c*8+o]
  where  Z2[v, c*8+o] = (x.reshape(V,KD) @ W.transpose(2,1,3,0).reshape(KD,CO))[v, c*8+o]
                        + b.sum(-1).T.reshape(CO)[c*8+o]

The dominant work is the [E,V] @ [V, CO] GEMM over the 256 MB boundary
operator G (68.7 GFLOP); Z2 is a 4.3 GFLOP preprocessing folded on host.
Sharding (per hint): G and out row-wise over E across 8 cores (data
parallel over out-simplices); Z2 (8 MB bf16) replicated; no collectives.
G ships as bf16 lhsT (host transpose+cast) so the TensorE runs at its
78.6 TF/s bf16 peak; accumulation is fp32 in PSUM (rel err ~2.3e-3).
fp8 was evaluated and rejected: e4m3 quantization of G/Z2 gives ~3.7%
rel err (gate is 2e-2) and the error-compensated 3-term variant costs
1.5x the bf16 matmul stream.

SYNCHRONIZATION (the part that was subtly wrong before): a DMA's
`.then_inc(sem, 16)` is performed as SIXTEEN independent +1 increments,
one from each SDMA engine as it finishes its own 8-partition share.
With several DMAs in flight on one semaphore, increments from a later
DMA can satisfy an earlier DMA's cumulative threshold while one lagging
SDMA engine still owes its 8 partitions -> the consumer reads 8 stale
partition rows (observed as rank-8 corruption under NTFF profiling,
which skews SDMA engine progress). Therefore every DMA that a consumer
waits on gets its OWN single-use semaphore (threshold 16 == fully
landed); only the final all-DMAs-done wait (s_out) uses a cumulative
count, which is sound because the final total requires every increment.

Device kernel (per core, E_loc = 1024 rows), hand-scheduled nc.Block:
  - GT and Z2 are shipped partition-major ([128, 64, cols]) so every
    descriptor moves 2-8 KB contiguous per partition.
  - Z2 [128, 64, 512] bf16 resident in SBUF on the scalar HWDGE queue;
    groups past the ramp are paced by PE chunk consumption (s_mm) to
    keep the early DMA burst from starving the PE's weight prefetch
    (unpaced, the first ~12 matmuls ran at 2x time).
  - GT chunks [128, 1024] bf16 stream through a 16-slot ring on the sync
    HWDGE queue (singles/pairs through the ramp, then quads).
  - 512 bf16 matmuls accumulate into all 8 PSUM banks (216 ns/mm =
    silicon limit for N=512).
  - Tail: VectorE copies PSUM tiles 0-3 to SBUF fp16 (SyncE ships them
    in pairs), ScalarE copies+ships tiles 4-7. Out is fp16; the host
    upcasts to f32 (adds ~0.05% error against a 2e-2 gate).

Timing on the 8-core axon TRN2 (fast clock mode): ~130.5-133 us HW
exec = ~11 us preamble (framework all-engine barrier waits ~6 us for
the GpSimd Q7 boot, then first-chunk DMA + completion receipt) +
~113 us matmul stream (+0.5 us residual ramp/stall) + ~7 us tail
(last PSUM evac + out DMA receipt + NEFF completion detection). Some
processes land in a ~20% slower DVFS mode (~155 us) regardless of
kernel content. Two further experiments were tried and REVERTED:
issuing the first DMAs before the framework preamble barrier (races
the runtime's exec-start DMA init on the first execution -> scattered
garbage), and a reordered v=63 tail with minimal copy thresholds
(intermittently shipped stale tail columns of e-tile 7).
"""

import numpy as np
import ml_dtypes

V = 8192
E = 8192
K = 64
C = 64
D = 8
O = 8
KD = K * D    # 512
CO = C * O    # 512
N_CORES = 8
EL = E // N_CORES  # 1024 out-rows per core
N_VCHUNK = V // 128  # 64
N_ETILE = EL // 128  # 8

BF16 = ml_dtypes.bfloat16

_cache = {}

# GT dma plan: chunks 0 and 1 alone (so the PE can start as soon as 256 KB
# lands), pairs through the ramp, then quads (8 KB contiguous per
# partition per descriptor). The first N_GT_PRE dmas are issued before
# the preamble barrier.
GT_DMAS = [(0, 1), (1, 1), (2, 2), (4, 2), (6, 2)] + [
    (8 + 4 * t, 4) for t in range((N_VCHUNK - 8) // 4)
]
assert sum(n for _, n in GT_DMAS) == N_VCHUNK
N_GT_PRE = 3   # chunks 0-3
_GT_IDX = {}
for _d, (_a, _n) in enumerate(GT_DMAS):
    for _c in range(_a, _a + _n):
        _GT_IDX[_c] = _d

# z2 load groups: small first so the v=0 matmuls start ASAP and the
# ramp chunks (4-7) are not stuck behind a 1 MB transfer (8-chunk
# groups were tried past the ramp: fewer PE wait-crossings but lumpier
# delivery, net ~0.3-0.5 us slower than quads)
Z2_GROUPS = [1, 1, 2, 2, 2] + [4] * 14
assert sum(Z2_GROUPS) == N_VCHUNK
N_Z2_PRE = 3   # chunks 0-3

# et emission order for the final (v=63) chunk. A reordered tail
# ([4,5,6,7,0,1,2,3] with minimal copy thresholds) shaved ~1 us but
# intermittently shipped garbage tail columns of e-tile 7 on first
# executions (copy/DMA racing data visibility while the PE is still
# active); the plain order with each copy gated on the FULL v=63 set
# finishing has soaked clean.
FIN_ORDER = list(range(N_ETILE))
FIN_THRESH = {et: et + 1 for et in FIN_ORDER}


def _gt_dma_idx(v):
    """Index of the GT dma that carries chunk v."""
    return _GT_IDX[v]


def _build_bass_raw():
    import os
    import concourse.mybir as mybir
    from concourse import bacc

    f32 = mybir.dt.float32
    fp16 = mybir.dt.float16
    bf16 = mybir.dt.bfloat16

    nc = bacc.Bacc("TRN2", target_bir_lowering=False)

    # partition-major layouts prepared on host
    gt = nc.dram_tensor("gt", (128, N_VCHUNK, EL), bf16, kind="ExternalInput")
    z2 = nc.dram_tensor("z2", (128, N_VCHUNK, CO), bf16, kind="ExternalInput")
    out = nc.dram_tensor("out", (128, N_ETILE, CO), fp16, kind="ExternalOutput")

    NSLOT = 16  # gt ring depth (chunks of [128, EL] bf16, 2KB/partition each)
    z2sb = nc.alloc_sbuf_tensor("z2sb", [128, N_VCHUNK, CO], bf16)
    gtsb = nc.alloc_sbuf_tensor("gtsb", [128, NSLOT, EL], bf16)
    osb = nc.alloc_sbuf_tensor("osb", [128, N_ETILE, CO], fp16)
    ps = [nc.alloc_psum_tensor(f"ps{i}", [128, CO], f32) for i in range(N_ETILE)]

    # single-use DMA-completion sems (see module docstring)
    gts = [nc.alloc_semaphore(f"s_gt{d}") for d in range(len(GT_DMAS))]
    # chunks 0 and 1 ship as column halves so the PE can start on
    # e-tiles 0-3 after only 128 KB lands; gts[d] covers cols 0-511,
    # s_g0b/s_g1b the rest (the unsplit chunk 1 stalled the PE ~0.8 us)
    s_g0b = nc.alloc_semaphore("s_g0b")
    s_g1b = nc.alloc_semaphore("s_g1b")
    z2s = [nc.alloc_semaphore(f"s_z2{g}") for g in range(len(Z2_GROUPS))]
    s_mm = nc.alloc_semaphore("s_mm")    # PE consumed chunk v: v+1
    s_fin = nc.alloc_semaphore("s_fin")  # v=63 matmuls retired, FIN_ORDER
    s_cpv = nc.alloc_semaphore("s_cpv")  # DVE psum->sbuf copies done
    s_out = nc.alloc_semaphore("s_out")  # out DMAs landed: 16 each

    all_sems = gts + [s_g0b, s_g1b] + z2s + [s_mm, s_fin, s_cpv, s_out]
    nums = [s.num for s in all_sems]
    assert nums == list(range(nums[0], nums[0] + len(nums))), nums
    sem_range = range(nums[0], nums[-1] + 1)
    # cleared at start: only sems without pre-barrier producers (the
    # gt/z2 sems rely on NEFF-load zeroing + the end-of-kernel clear)
    sem_range_start = range(s_mm.num, s_out.num + 1)

    groups = []
    v0 = 0
    for zg in Z2_GROUPS:
        groups.append((v0, zg))
        v0 += zg

    def gt_dma(eng, d):
        a, n = GT_DMAS[d]
        sl = a % NSLOT
        eng.dma_start(gtsb[:, sl:sl + n, :], gt[:, a:a + n, :]).then_inc(
            gts[d], 16
        )

    def z2_dma(eng, g):
        v0g, zg = groups[g]
        eng.dma_start(
            z2sb[:, v0g:v0g + zg, :], z2[:, v0g:v0g + zg, :]
        ).then_inc(z2s[g], 16)

    # Mid-preamble issue of the first dmas: inserted AFTER the framework's
    # const-tile memsets (~6.0 us into the preamble, past the runtime's
    # ~2.5 us exec-start DMA-init race window that corrupted the old
    # before-the-first-barrier variant) but BEFORE the final all-engine
    # barrier. Both go on Scalar, the engine that reaches this point
    # earliest (~6.05 us vs Sync's ~6.8), so the ~0.7 us/descriptor issue
    # cost delays the barrier the least. First-chunk flight (1.5 us queue
    # kick + transfer + completion receipt ~ 3 us) then overlaps the
    # barrier + branch instead of starting after them.
    use_midpre = os.environ.get("KOPT_MIDPRE", "1") != "0"
    if use_midpre:
        entry = nc.main_func.blocks[0]
        pre_n = len(entry.instructions)
        z2_dma(nc.scalar, 0)
        nc.scalar.dma_start(gtsb[:, 0, 0:512], gt[:, 0, 0:512]).then_inc(
            gts[0], 16
        )
        mine = entry.instructions[pre_n:]
        assert len(mine) == 2, len(mine)
        del entry.instructions[pre_n:]
        anchor = 1 + max(
            i for i, ins in enumerate(entry.instructions)
            if type(ins).__name__ == "InstMemset"
        )
        for off, ins in enumerate(mine):
            entry.instructions.insert(anchor + off, ins)
    use_nowait = os.environ.get("KOPT_NOWAIT", "1") != "0"
    use_tail = os.environ.get("KOPT_TAIL", "1") != "0"

    with nc.Block(name="k", no_gpsimd_drain=True) as blk:

        @blk.sync
        def _(eng):
            eng.sem_clear(sem_range_start)
            for d in range(len(GT_DMAS)):
                a, n = GT_DMAS[d]
                if a >= NSLOT:
                    eng.wait_ge(s_mm, a + n - NSLOT)
                elif a >= 8:
                    # pace the ramp: chunks 8-15 are not needed for
                    # ~14 us; issuing them at exec start contributes to
                    # the DMA burst that starves the PE weight prefetch
                    # (first ~12 matmuls ran at 2x time)
                    eng.wait_ge(s_mm, a - 7)
                if d in (0, 1):
                    half = s_g0b if d == 0 else s_g1b
                    if not (d == 0 and use_midpre):
                        # chunk 0 cols 0:512 already issued mid-preamble
                        eng.dma_start(
                            gtsb[:, d, 0:512], gt[:, d, 0:512]
                        ).then_inc(gts[d], 16)
                    eng.dma_start(
                        gtsb[:, d, 512:EL], gt[:, d, 512:EL]
                    ).then_inc(half, 16)
                else:
                    gt_dma(eng, d)
            for k, et in enumerate((0, 2)):
                eng.wait_ge(s_cpv, 2 * (k + 1))
                eng.dma_start(
                    out[:, et:et + 2, :], osb[:, et:et + 2, :]
                ).then_inc(s_out, 16)
            if use_tail:
                # e-tile 7 evacuated by DVE (5th s_cpv inc), shipped here
                eng.wait_ge(s_cpv, 5)
                eng.dma_start(
                    out[:, 7:8, :], osb[:, 7:8, :]
                ).then_inc(s_out, 16)
            if not use_nowait:
                eng.wait_ge(s_out, 16 * 5)
            # The final all-DMAs-landed wait is dropped (KOPT_NOWAIT): the
            # runtime's end-of-NEFF epilogue (an all-engine barrier + ~6.9 us
            # of per-engine semaphore clears + a final barrier, appended by
            # the runtime after this program) runs before the host can
            # observe completion, and the last out DMA's receipt (~3.2 us)
            # lands well inside it. Late s_out increments landing after the
            # range-clear below are wiped by the start-of-block clear on the
            # next execution.
            # leave sems zeroed so a re-execution of the loaded NEFF works
            eng.sem_clear(sem_range)

        @blk.scalar
        def _(eng):
            for g in range(1 if use_midpre else 0, len(groups)):
                c0 = groups[g][0]
                if 8 <= c0 < 12:
                    eng.wait_ge(s_mm, 1)
                elif c0 >= 12:
                    # pace Z2 groups by PE consumption: a group starting
                    # at chunk c0 issues ~10 consumed chunks (~17 us)
                    # before it is needed, keeping the early DMA burst
                    # from starving the PE's weight prefetch during the
                    # ramp (which ran the first ~12 matmuls at 2x time;
                    # pacing the ramp groups as well starved chunk 4)
                    eng.wait_ge(s_mm, c0 - 10)
                z2_dma(eng, g)
            sc_tiles = (4, 5, 6) if use_tail else (4, 5, 6, 7)
            for et in sc_tiles:
                eng.wait_ge(s_fin, FIN_THRESH[et])
                eng.copy(osb[:, et, :], ps[et][:])
                if et == 5:
                    eng.dma_start(
                        out[:, 4:6, :], osb[:, 4:6, :]
                    ).then_inc(s_out, 16)
                elif et >= 6:
                    # tiles 6 and 7 ship as singles: the final,
                    # receipt-latency-bound dma is 128 KB instead of 256
                    eng.dma_start(
                        out[:, et:et + 1, :], osb[:, et:et + 1, :]
                    ).then_inc(s_out, 16)

        @blk.tensor
        def _(eng):
            # NOTE: PE-warmup matmuls were tried here to absorb the
            # 2x-slow first ~12 matmuls (confirmed to be PE warm-up, not
            # data starvation) — but the fast stream then just stalls on
            # GT chunk delivery instead: the ramp is delivery-bound and
            # the slow window hides it for free. Reverted as neutral.
            landed = 0
            g = 0
            for v in range(N_VCHUNK):
                while v >= landed:
                    eng.wait_ge(z2s[g], 16)
                    landed += groups[g][1]
                    g += 1
                d = _gt_dma_idx(v)
                if v == GT_DMAS[d][0]:
                    # one wait per GT dma (chunks of the same dma share a
                    # completion sem; redundant waits cost PE issue time)
                    eng.wait_ge(gts[d], 16)
                last = v == N_VCHUNK - 1
                ets = FIN_ORDER if last else range(N_ETILE)
                for j, et in enumerate(ets):
                    if v in (0, 1) and et == 4:
                        eng.wait_ge(s_g0b if v == 0 else s_g1b, 16)
                    mm = eng.matmul(
                        ps[et][:],
                        lhsT=gtsb[:, v % NSLOT, et * 128:(et + 1) * 128],
                        rhs=z2sb[:, v, :],
                        start=(v == 0),
                        stop=last,
                    )
                    if j == N_ETILE - 1 and not last:
                        mm.then_inc(s_mm, 1)
                    if last:
                        mm.then_inc(s_fin, 1)

        @blk.vector
        def _(eng):
            # et7 rides on DVE (KOPT_TAIL): after its 4 early-tile copies
            # the DVE is idle while the Scalar engine serially evacuates
            # 4,5,6,7; moving the last (critical-path) tile here overlaps
            # it with Scalar's et6 work. Sync ships it on the 5th s_cpv.
            ve_tiles = (0, 1, 2, 3, 7) if use_tail else (0, 1, 2, 3)
            for et in ve_tiles:
                eng.wait_ge(s_fin, FIN_THRESH[et])
                eng.tensor_copy(osb[:, et, :], ps[et][:]).then_inc(s_cpv, 1)

    nc.compile()
    return nc


def _prep_inputs(x, G, W, b):
    x = np.asarray(x, dtype=np.float32)
    G = np.asarray(G, dtype=np.float32)
    W = np.asarray(W, dtype=np.float32)
    b = np.asarray(b, dtype=np.float32)

    X2 = np.ascontiguousarray(x.reshape(V, KD))                 # [V, (k,d)]
    WM = np.ascontiguousarray(W.transpose(2, 1, 3, 0).reshape(KD, CO))  # [(k,d),(c,o)]
    bias = b.sum(axis=-1).T.reshape(CO)                          # [(c,o)]
    Z2 = (X2 @ WM + bias[None, :]).astype(BF16)                  # [V, CO]
    # partition-major: [128, n, CO], row v = n*128 + p
    Z2P = np.ascontiguousarray(Z2.reshape(N_VCHUNK, 128, CO).transpose(1, 0, 2))

    GT = G.T.astype(BF16)                                        # [V, E] contiguous
    in_maps = []
    for c in range(N_CORES):
        GTc = GT[:, c * EL:(c + 1) * EL]                          # [V, EL]
        GTP = np.ascontiguousarray(
            GTc.reshape(N_VCHUNK, 128, EL).transpose(1, 0, 2)    # [128, n, EL]
        )
        in_maps.append({"gt": GTP, "z2": Z2P})
    return in_maps


def _run(x, G, W, b, trace=False, trace_cores=None):
    from concourse.bass_utils import run_bass_kernel_spmd

    if "raw" not in _cache:
        _cache["raw"] = _build_bass_raw()
    nc = _cache["raw"]

    in_maps = _prep_inputs(x, G, W, b)
    kw = {}
    if trace_cores is not None:
        kw["trace_cores"] = trace_cores
    res = run_bass_kernel_spmd(
        nc, in_maps, core_ids=list(range(N_CORES)), trace=trace, **kw,
    )
    # out is [128, 8, 512] fp16 per core, row e_loc = et*128 + p
    outs = []
    for c in range(N_CORES):
        o = res.results[c]["out"]
        outs.append(np.ascontiguousarray(o.transpose(1, 0, 2)).reshape(EL, CO))
    out = np.concatenate(outs, axis=0).astype(np.float32)
    out = out.reshape(E, C, O)
    return out, res


def kernel(x, G, W, b):
    out, _ = _run(x, G, W, b, trace=False)
    return out



# revision 8
# speedup vs baseline: 1.1175x; 1.1058x over previous
"""Trainium2 Bass kernel for nn_BlockLTN (gnn_message_passing).

Math:
    z[o,v,c] = sum_{k,d} x[v,k,d] * W[o,d,k,c] + sum_d b[o,c,d]
    out[e,c,o] = sum_v G[e,v] * z[o,v,c]

Folded:  out[e, c*8+o] = G[e,:] @ Z2[:, c*8+o]
  where  Z2[v, c*8+o] = (x.reshape(V,KD) @ W.transpose(2,1,3,0).reshape(KD,CO))[v, c*8+o]
                        + b.sum(-1).T.reshape(CO)[c*8+o]

The dominant work is the [E,V] @ [V, CO] GEMM over the 256 MB boundary
operator G (68.7 GFLOP); Z2 is a 4.3 GFLOP preprocessing folded on host.
Sharding (per hint): G and out row-wise over E across 8 cores (data
parallel over out-simplices); Z2 replicated; no collectives.

MIXED PRECISION: the first N8=14 v-chunks (1792 of 8192 contraction
rows, 21.9%) run as 7 fp8-e4m3 DoubleRow matmuls per e-tile (2 weights
per PE cell, 2 MACs/cycle); the remaining 50 chunks run bf16 at the
1 column/cycle silicon limit (216 ns per 128x128x512 MM). Error budget:
e4m3 quantization of both operands costs ~3.7% rel err at full
coverage; at 21.9% coverage the measured host-sim error is 1.76e-2 vs
the 2e-2 gate (bf16-only was 2.35e-3). Scaling: G ships x128 and Z2 x8
(exact powers of two, applied to BOTH precisions so the mixed PSUM
accumulation is uniform; G*128 lifts sigma to 2.56 so values clear the
e4m3 min-normal 0.015625 -- unscaled, 56% of G is subnormal garbage);
the host divides the fp16 output by 1024 (max |out*1024| ~ 27k < 65504).
fp8 pairs run FIRST: the HAM-cold window (~3.4 us at 1.2 GHz from
kernel start) covers work at 2 MACs/cycle, and the delivery-bound ramp
needs half the DMA bytes.

SYNCHRONIZATION (inherited, load-bearing): a DMA's `.then_inc(sem, 16)`
is performed as SIXTEEN independent +1 increments, one per SDMA engine.
With several DMAs in flight on one semaphore, a later DMA's increments
can satisfy an earlier DMA's cumulative threshold while one lagging
SDMA engine still owes its 8 partitions -> rank-8 stale-data corruption
(observed under NTFF profiling skew). Therefore every DMA a consumer
waits on gets its OWN single-use semaphore (threshold 16 == fully
landed).

Tail (v=63, bf16, unchanged order 0..7 -- a reordered tail shipped
garbage intermittently in a prior session): Vector evacuates e-tiles
0-3 and 7 (PSUM->SBUF fp16), Scalar 4-6; Sync ships 0:2, 2:4 and 7,
Scalar ships 4:6, 6. The final all-DMAs-landed wait (s_out) is DROPPED:
the runtime's end-of-NEFF epilogue (all-engine barrier + ~6.9 us of
per-engine semaphore clears + final barrier, appended by the runtime
after this program) runs before the host can observe completion, and
the last out-DMA receipt (~3.2 us) lands well inside it. s_out residue
from late receipts is wiped by the start-of-block clear on the next
execution.

Preamble: the profiler's exec-time clock starts at the framework's
const-tile MEMSETs (~6.05 us into the runtime preamble); the two DMAs
gating the first matmul (z28 chunks 0:2 and gt8 chunks 0:2 cols 0:512)
are relocated into the preamble right after those MEMSETs on the
Scalar queue -- past the runtime's ~2.5 us exec-start DMA-init race
window that corrupted a start-of-preamble variant, but ~1.2 us before
the all-engine barrier releases.
"""

import os

import numpy as np
import ml_dtypes

V = 8192
E = 8192
K = 64
C = 64
D = 8
O = 8
KD = K * D    # 512
CO = C * O    # 512
N_CORES = 8
EL = E // N_CORES  # 1024 out-rows per core
N_VCHUNK = V // 128  # 64
N_ETILE = EL // 128  # 8

N8 = 14               # fp8 chunks (v 0..13) -> 7 DoubleRow pairs
NP8 = N8 // 2
NB = N_VCHUNK - N8    # 50 bf16 chunks (v 14..63)
NSLOT = 16            # bf16 gt ring depth
LAM = 128.0           # G scale (2**7)
MU = 8.0              # Z2 scale (2**3)

BF16 = ml_dtypes.bfloat16
F8E4 = ml_dtypes.float8_e4m3  # TRN fp8_exp4: max +-240

# fp8 GT dma plan, in fp8-chunk spans. dma 0 ships chunks 0:2 as column
# halves (0:512 pre-issued mid-preamble on Scalar; 512:EL first in-block
# on Sync) so the PE can start on e-tiles 0-3 after 128 KB lands.
GT8_PLAN = [(0, 2), (2, 2), (4, 4), (8, 4), (12, 2)]
N_GT8_DMAS = len(GT8_PLAN) + 1  # +1 for the second column half of span 0
Z28_GROUPS = [(0, 2), (2, 2), (4, 4), (8, 6)]  # first group pre-issued

# bf16 GT dmas (chunks 14..63) through the 16-slot ring, slot (a-14)%16.
# Spans are chosen so no dma wraps the ring boundary.
GT_DMAS = [(14, 2), (16, 2), (18, 2), (20, 4), (24, 4), (28, 2)] + [
    (30 + 4 * t, 4) for t in range(8)
] + [(62, 2)]
assert sum(n for _, n in GT_DMAS) == NB
for _a, _n in GT_DMAS:
    assert (_a - N8) % NSLOT + _n <= NSLOT, (_a, _n)
_GT_IDX = {}
for _d, (_a, _n) in enumerate(GT_DMAS):
    for _c in range(_a, _a + _n):
        _GT_IDX[_c] = _d

Z2_GROUPS = [(14, 2), (16, 2), (18, 2)] + [(20 + 4 * t, 4) for t in range(11)]
assert sum(n for _, n in Z2_GROUPS) == NB

# v=63 e-tile emission order; each final MM bumps s_fin.
FIN_ORDER = list(range(N_ETILE))
FIN_THRESH = {et: et + 1 for et in FIN_ORDER}


def _build_bass_raw():
    import concourse.mybir as mybir
    from concourse import bacc

    f32 = mybir.dt.float32
    fp16 = mybir.dt.float16
    bf16 = mybir.dt.bfloat16
    f8e4 = mybir.dt.float8e4
    DR = mybir.MatmulPerfMode.DoubleRow

    nc = bacc.Bacc("TRN2", target_bir_lowering=False)

    # partition-major layouts prepared on host
    gt8 = nc.dram_tensor("gt8", (128, N8, EL), f8e4, kind="ExternalInput")
    z28 = nc.dram_tensor("z28", (128, N8, CO), f8e4, kind="ExternalInput")
    gt = nc.dram_tensor("gt", (128, NB, EL), bf16, kind="ExternalInput")
    z2 = nc.dram_tensor("z2", (128, NB, CO), bf16, kind="ExternalInput")
    out = nc.dram_tensor("out", (128, N_ETILE, CO), fp16, kind="ExternalOutput")

    gtsb8 = nc.alloc_sbuf_tensor("gtsb8", [128, N8, EL], f8e4)
    z2sb8 = nc.alloc_sbuf_tensor("z2sb8", [128, N8, CO], f8e4)
    gtsb = nc.alloc_sbuf_tensor("gtsb", [128, NSLOT, EL], bf16)
    z2sb = nc.alloc_sbuf_tensor("z2sb", [128, NB, CO], bf16)
    osb = nc.alloc_sbuf_tensor("osb", [128, N_ETILE, CO], fp16)
    ps = [nc.alloc_psum_tensor(f"ps{i}", [128, CO], f32) for i in range(N_ETILE)]

    # single-use DMA-completion sems (see module docstring)
    g8s = [nc.alloc_semaphore(f"s_g8{d}") for d in range(N_GT8_DMAS)]
    z28s = [nc.alloc_semaphore(f"s_z28{g}") for g in range(len(Z28_GROUPS))]
    gts = [nc.alloc_semaphore(f"s_gt{d}") for d in range(len(GT_DMAS))]
    z2s = [nc.alloc_semaphore(f"s_z2{g}") for g in range(len(Z2_GROUPS))]
    s_mm = nc.alloc_semaphore("s_mm")    # chunks consumed by the PE
    s_fin = nc.alloc_semaphore("s_fin")  # v=63 matmuls retired, FIN_ORDER
    s_cpv = nc.alloc_semaphore("s_cpv")  # DVE psum->sbuf copies done
    s_out = nc.alloc_semaphore("s_out")  # out DMAs landed: 16 each

    all_sems = g8s + z28s + gts + z2s + [s_mm, s_fin, s_cpv, s_out]
    nums = [s.num for s in all_sems]
    assert nums == list(range(nums[0], nums[0] + len(nums))), nums
    sem_range = range(nums[0], nums[-1] + 1)
    # cleared at start: only sems without pre-barrier producers (the
    # gt/z2 sems rely on NEFF-load zeroing + the end-of-kernel clear)
    sem_range_start = range(s_mm.num, s_out.num + 1)

    def z28_dma(eng, g):
        a, n = Z28_GROUPS[g]
        eng.dma_start(z2sb8[:, a:a + n, :], z28[:, a:a + n, :]).then_inc(
            z28s[g], 16
        )

    use_midpre = os.environ.get("KOPT_MIDPRE", "1") != "0"
    use_tail = os.environ.get("KOPT_TAIL", "1") != "0"
    use_nowait = os.environ.get("KOPT_NOWAIT", "1") != "0"

    if use_midpre:
        # Relocate the two first-matmul-gating DMAs to just after the
        # framework's const-tile memsets (~6.05 us in, past the runtime's
        # ~2.5 us DMA-init race window, before the all-engine barrier).
        entry = nc.main_func.blocks[0]
        pre_n = len(entry.instructions)
        nc.scalar.dma_start(
            gtsb8[:, 0:2, 0:512], gt8[:, 0:2, 0:512]
        ).then_inc(g8s[0], 16)
        z28_dma(nc.scalar, 0)
        mine = entry.instructions[pre_n:]
        assert len(mine) == 2, len(mine)
        del entry.instructions[pre_n:]
        anchor = 1 + max(
            i for i, ins in enumerate(entry.instructions)
            if type(ins).__name__ == "InstMemset"
        )
        for off, ins in enumerate(mine):
            entry.instructions.insert(anchor + off, ins)

    with nc.Block(name="k", no_gpsimd_drain=True) as blk:

        @blk.sync
        def _(eng):
            eng.sem_clear(sem_range_start)
            if not use_midpre:
                eng.dma_start(
                    gtsb8[:, 0:2, 0:512], gt8[:, 0:2, 0:512]
                ).then_inc(g8s[0], 16)
            # second column half of fp8 chunks 0:2 (gates e-tiles 4-7)
            eng.dma_start(
                gtsb8[:, 0:2, 512:EL], gt8[:, 0:2, 512:EL]
            ).then_inc(g8s[1], 16)
            for i, (a, n) in enumerate(GT8_PLAN[1:]):
                if a >= 8:
                    # pace the late fp8 spans by PE pair consumption
                    eng.wait_ge(s_mm, a - 6)
                eng.dma_start(
                    gtsb8[:, a:a + n, :], gt8[:, a:a + n, :]
                ).then_inc(g8s[2 + i], 16)
            for d, (a, n) in enumerate(GT_DMAS):
                if a >= 30:
                    # ring-slot reuse: chunk a lands in the slot chunk
                    # a-16 occupied
                    eng.wait_ge(s_mm, a + n - NSLOT)
                elif a >= 18:
                    # burst pacing through the ramp
                    eng.wait_ge(s_mm, a - 9)
                sl = (a - N8) % NSLOT
                eng.dma_start(
                    gtsb[:, sl:sl + n, :], gt[:, a - N8:a - N8 + n, :]
                ).then_inc(gts[d], 16)
            for k, et in enumerate((0, 2)):
                eng.wait_ge(s_cpv, 2 * (k + 1))
                eng.dma_start(
                    out[:, et:et + 2, :], osb[:, et:et + 2, :]
                ).then_inc(s_out, 16)
            if use_tail:
                # e-tile 7 evacuated by DVE (5th s_cpv inc), shipped here
                eng.wait_ge(s_cpv, 5)
                eng.dma_start(
                    out[:, 7:8, :], osb[:, 7:8, :]
                ).then_inc(s_out, 16)
            if not use_nowait:
                eng.wait_ge(s_out, 16 * 5)
            # leave sems zeroed so a re-execution of the loaded NEFF works
            eng.sem_clear(sem_range)

        @blk.scalar
        def _(eng):
            for g in range(0 if not use_midpre else 1, len(Z28_GROUPS)):
                if Z28_GROUPS[g][0] >= 8:
                    eng.wait_ge(s_mm, Z28_GROUPS[g][0] - 6)
                z28_dma(eng, g)
            for g, (c0, n) in enumerate(Z2_GROUPS):
                if c0 >= 18:
                    # pace Z2 groups by PE consumption (see prior session:
                    # an unpaced burst starved the PE weight prefetch)
                    eng.wait_ge(s_mm, c0 - 10)
                eng.dma_start(
                    z2sb[:, c0 - N8:c0 - N8 + n, :], z2[:, c0 - N8:c0 - N8 + n, :]
                ).then_inc(z2s[g], 16)
            sc_tiles = (4, 5, 6) if use_tail else (4, 5, 6, 7)
            for et in sc_tiles:
                eng.wait_ge(s_fin, FIN_THRESH[et])
                eng.copy(osb[:, et, :], ps[et][:])
                if et == 5:
                    eng.dma_start(
                        out[:, 4:6, :], osb[:, 4:6, :]
                    ).then_inc(s_out, 16)
                elif et >= 6:
                    eng.dma_start(
                        out[:, et:et + 1, :], osb[:, et:et + 1, :]
                    ).then_inc(s_out, 16)

        @blk.tensor
        def _(eng):
            # fp8 DoubleRow pairs first (chunks 0..13)
            landed8 = 0
            g8 = 0
            for t in range(NP8):
                while landed8 < 2 * t + 2:
                    eng.wait_ge(z28s[g8], 16)
                    landed8 += Z28_GROUPS[g8][1]
                    g8 += 1
                # one wait per gt8 dma, on its first consuming pair
                if t == 0:
                    eng.wait_ge(g8s[0], 16)
                else:
                    for i, (a, n) in enumerate(GT8_PLAN[1:]):
                        if 2 * t == a:
                            eng.wait_ge(g8s[2 + i], 16)
                for j, et in enumerate(range(N_ETILE)):
                    if t == 0 and et == 4:
                        eng.wait_ge(g8s[1], 16)
                    mm = eng.matmul(
                        ps[et][:],
                        lhsT=gtsb8[:, 2 * t:2 * t + 2, et * 128:(et + 1) * 128],
                        rhs=z2sb8[:, 2 * t:2 * t + 2, :],
                        start=(t == 0),
                        stop=False,
                        perf_mode=DR,
                    )
                    if j == N_ETILE - 1:
                        mm.then_inc(s_mm, 2)
            # bf16 chunks 14..63
            landed = N8
            g = 0
            for v in range(N8, N_VCHUNK):
                while v >= landed:
                    eng.wait_ge(z2s[g], 16)
                    landed += Z2_GROUPS[g][1]
                    g += 1
                d = _GT_IDX[v]
                if v == GT_DMAS[d][0]:
                    eng.wait_ge(gts[d], 16)
                last = v == N_VCHUNK - 1
                sl = (v - N8) % NSLOT
                ets = FIN_ORDER if last else range(N_ETILE)
                for j, et in enumerate(ets):
                    mm = eng.matmul(
                        ps[et][:],
                        lhsT=gtsb[:, sl, et * 128:(et + 1) * 128],
                        rhs=z2sb[:, v - N8, :],
                        start=False,
                        stop=last,
                    )
                    if j == N_ETILE - 1 and not last:
                        mm.then_inc(s_mm, 1)
                    if last:
                        mm.then_inc(s_fin, 1)

        @blk.vector
        def _(eng):
            # et7 rides on DVE: after its 4 early-tile copies the DVE is
            # idle while Scalar serially evacuates 4,5,6; the last
            # (critical-path) tile overlaps Scalar's et6 work. Sync ships
            # it on the 5th s_cpv.
            ve_tiles = (0, 1, 2, 3, 7) if use_tail else (0, 1, 2, 3)
            for et in ve_tiles:
                eng.wait_ge(s_fin, FIN_THRESH[et])
                eng.tensor_copy(osb[:, et, :], ps[et][:]).then_inc(s_cpv, 1)

    nc.compile()
    return nc


_cache = {}


def _prep_inputs(x, G, W, b):
    x = np.asarray(x, dtype=np.float32)
    G = np.asarray(G, dtype=np.float32)
    W = np.asarray(W, dtype=np.float32)
    b = np.asarray(b, dtype=np.float32)

    X2 = np.ascontiguousarray(x.reshape(V, KD))                 # [V, (k,d)]
    WM = np.ascontiguousarray(W.transpose(2, 1, 3, 0).reshape(KD, CO))
    bias = b.sum(axis=-1).T.reshape(CO)                          # [(c,o)]
    Z2 = (X2 @ WM + bias[None, :]) * MU                          # [V, CO] scaled

    VS8 = N8 * 128  # 1792 fp8 contraction rows
    # fp8 part, partition-major [128, N8, CO]
    Z28P = np.ascontiguousarray(
        np.clip(Z2[:VS8], -240, 240)
        .astype(F8E4)
        .reshape(N8, 128, CO)
        .transpose(1, 0, 2)
    )
    Z2P = np.ascontiguousarray(
        Z2[VS8:].astype(BF16).reshape(NB, 128, CO).transpose(1, 0, 2)
    )

    GT = G.T * LAM                                               # [V, E] scaled
    GT8 = np.clip(GT[:VS8], -240, 240).astype(F8E4)
    GTB = GT[VS8:].astype(BF16)
    in_maps = []
    for c in range(N_CORES):
        sl = slice(c * EL, (c + 1) * EL)
        GT8P = np.ascontiguousarray(
            GT8[:, sl].reshape(N8, 128, EL).transpose(1, 0, 2)
        )
        GTP = np.ascontiguousarray(
            GTB[:, sl].reshape(NB, 128, EL).transpose(1, 0, 2)
        )
        in_maps.append({"gt8": GT8P, "z28": Z28P, "gt": GTP, "z2": Z2P})
    return in_maps


def _run(x, G, W, b, trace=False, trace_cores=None):
    from concourse.bass_utils import run_bass_kernel_spmd

    if "raw" not in _cache:
        _cache["raw"] = _build_bass_raw()
    nc = _cache["raw"]

    in_maps = _prep_inputs(x, G, W, b)
    kw = {}
    if trace_cores is not None:
        kw["trace_cores"] = trace_cores
    res = run_bass_kernel_spmd(
        nc, in_maps, core_ids=list(range(N_CORES)), trace=trace, **kw,
    )
    # out is [128, 8, 512] fp16 per core (scaled by LAM*MU), row
    # e_loc = et*128 + p
    outs = []
    for c in range(N_CORES):
        o = res.results[c]["out"]
        outs.append(np.ascontiguousarray(o.transpose(1, 0, 2)).reshape(EL, CO))
    out = np.concatenate(outs, axis=0).astype(np.float32) * (1.0 / (LAM * MU))
    out = out.reshape(E, C, O)
    return out, res


def kernel(x, G, W, b):
    out, _ = _run(x, G, W, b, trace=False)
    return out


# revision 13
# speedup vs baseline: 1.1296x; 1.0109x over previous
"""Trainium2 Bass kernel for nn_BlockLTN (gnn_message_passing).

Math:
    z[o,v,c] = sum_{k,d} x[v,k,d] * W[o,d,k,c] + sum_d b[o,c,d]
    out[e,c,o] = sum_v G[e,v] * z[o,v,c]

Folded:  out[e, c*8+o] = G[e,:] @ Z2[:, c*8+o]
  where  Z2[v, c*8+o] = (x.reshape(V,KD) @ W.transpose(2,1,3,0).reshape(KD,CO))[v, c*8+o]
                        + b.sum(-1).T.reshape(CO)[c*8+o]

The dominant work is the [E,V] @ [V, CO] GEMM over the 256 MB boundary
operator G (68.7 GFLOP); Z2 is a 4.3 GFLOP preprocessing folded on host.
Sharding (per hint): G and out row-wise over E across 8 cores (data
parallel over out-simplices); Z2 replicated; no collectives.

MIXED PRECISION: the first N8=14 v-chunks (1792 of 8192 contraction
rows, 21.9%) run as 7 fp8-e4m3 DoubleRow matmuls per e-tile (2 weights
per PE cell, 2 MACs/cycle); the remaining 50 chunks run bf16 at the
1 column/cycle silicon limit (216 ns per 128x128x512 MM). Error budget:
e4m3 quantization of both operands costs ~3.7% rel err at full
coverage; at 21.9% coverage the measured host-sim error is 1.76e-2 vs
the 2e-2 gate (bf16-only was 2.35e-3). Scaling: G ships x128 and Z2 x8
(exact powers of two, applied to BOTH precisions so the mixed PSUM
accumulation is uniform; G*128 lifts sigma to 2.56 so values clear the
e4m3 min-normal 0.015625 -- unscaled, 56% of G is subnormal garbage);
the host divides the fp16 output by 1024 (max |out*1024| ~ 27k < 65504).
fp8 pairs run FIRST: the HAM-cold window (~3.4 us at 1.2 GHz from
kernel start) covers work at 2 MACs/cycle, and the delivery-bound ramp
needs half the DMA bytes.

SYNCHRONIZATION (inherited, load-bearing): a DMA's `.then_inc(sem, 16)`
is performed as SIXTEEN independent +1 increments, one per SDMA engine.
With several DMAs in flight on one semaphore, a later DMA's increments
can satisfy an earlier DMA's cumulative threshold while one lagging
SDMA engine still owes its 8 partitions -> rank-8 stale-data corruption
(observed under NTFF profiling skew). Therefore every DMA a consumer
waits on gets its OWN single-use semaphore (threshold 16 == fully
landed).

Tail (v=63, bf16, unchanged order 0..7 -- a reordered tail shipped
garbage intermittently in a prior session): Vector evacuates e-tiles
0-3 and 7 (PSUM->SBUF fp16), Scalar 4-6; Sync ships 0:2, 2:4 and 7,
Scalar ships 4:6, 6. The final all-DMAs-landed wait (s_out) is DROPPED:
the runtime's end-of-NEFF epilogue (all-engine barrier + ~6.9 us of
per-engine semaphore clears + final barrier, appended by the runtime
after this program) runs before the host can observe completion, and
the last out-DMA receipt (~3.2 us) lands well inside it. s_out residue
from late receipts is wiped by the start-of-block clear on the next
execution.

Preamble: the profiler's exec-time clock starts at the framework's
const-tile MEMSETs (~6.05 us into the runtime preamble); the two DMAs
gating the first matmul (z28 chunks 0:2 and gt8 chunks 0:2 cols 0:512)
are relocated into the preamble right after those MEMSETs on the
Scalar queue -- past the runtime's ~2.5 us exec-start DMA-init race
window that corrupted a start-of-preamble variant, but ~1.2 us before
the all-engine barrier releases.
"""

import os

import numpy as np
import ml_dtypes

V = 8192
E = 8192
K = 64
C = 64
D = 8
O = 8
KD = K * D    # 512
CO = C * O    # 512
N_CORES = 8
EL = E // N_CORES  # 1024 out-rows per core
N_VCHUNK = V // 128  # 64
N_ETILE = EL // 128  # 8

N8 = 14               # fp8 chunks (v 0..13) -> 7 DoubleRow pairs
NP8 = N8 // 2
NB = N_VCHUNK - N8    # 50 bf16 chunks (v 14..63)
NSLOT = 16            # bf16 gt ring depth
LAM = 128.0           # G scale (2**7)
MU = 8.0              # Z2 scale (2**3)

BF16 = ml_dtypes.bfloat16
F8E4 = ml_dtypes.float8_e4m3  # TRN fp8_exp4: max +-240

# fp8 GT dma plan, in fp8-chunk spans. dma 0 (chunks 0:2, pre-issued
# mid-preamble on Scalar) ships WHOLE: a column split halves the
# per-partition contiguous run to 512 B and the descriptor overhead
# stretched the first flight 3.0 -> 4.3 us; unsplit it merges to 2 KB.
GT8_PLAN = [(0, 2), (2, 2), (4, 4), (8, 4), (12, 2)]
N_GT8_DMAS = len(GT8_PLAN)
Z28_GROUPS = [(0, 2), (2, 2), (4, 4), (8, 6)]  # first group pre-issued

# bf16 GT dmas (chunks 14..63) through the 16-slot ring, slot (a-14)%16.
# Spans are chosen so no dma wraps the ring boundary.
GT_DMAS = [(14, 2), (16, 2), (18, 2), (20, 4), (24, 4), (28, 2)] + [
    (30 + 4 * t, 4) for t in range(8)
] + [(62, 2)]
assert sum(n for _, n in GT_DMAS) == NB
for _a, _n in GT_DMAS:
    assert (_a - N8) % NSLOT + _n <= NSLOT, (_a, _n)
_GT_IDX = {}
for _d, (_a, _n) in enumerate(GT_DMAS):
    for _c in range(_a, _a + _n):
        _GT_IDX[_c] = _d

Z2_GROUPS = [(14, 2), (16, 2), (18, 2)] + [(20 + 4 * t, 4) for t in range(11)]
assert sum(n for _, n in Z2_GROUPS) == NB

# v=63 e-tile emission order; each final MM bumps s_fin.
FIN_ORDER = list(range(N_ETILE))
FIN_THRESH = {et: et + 1 for et in FIN_ORDER}


def _build_bass_raw():
    import concourse.mybir as mybir
    from concourse import bacc

    f32 = mybir.dt.float32
    fp16 = mybir.dt.float16
    bf16 = mybir.dt.bfloat16
    f8e4 = mybir.dt.float8e4
    DR = mybir.MatmulPerfMode.DoubleRow

    nc = bacc.Bacc("TRN2", target_bir_lowering=False)

    # partition-major layouts prepared on host
    gt8 = nc.dram_tensor("gt8", (128, N8, EL), f8e4, kind="ExternalInput")
    z28 = nc.dram_tensor("z28", (128, N8, CO), f8e4, kind="ExternalInput")
    gt = nc.dram_tensor("gt", (128, NB, EL), bf16, kind="ExternalInput")
    z2 = nc.dram_tensor("z2", (128, NB, CO), bf16, kind="ExternalInput")
    out = nc.dram_tensor("out", (128, N_ETILE, CO), fp16, kind="ExternalOutput")

    gtsb8 = nc.alloc_sbuf_tensor("gtsb8", [128, N8, EL], f8e4)
    z2sb8 = nc.alloc_sbuf_tensor("z2sb8", [128, N8, CO], f8e4)
    gtsb = nc.alloc_sbuf_tensor("gtsb", [128, NSLOT, EL], bf16)
    z2sb = nc.alloc_sbuf_tensor("z2sb", [128, NB, CO], bf16)
    osb = nc.alloc_sbuf_tensor("osb", [128, N_ETILE, CO], fp16)
    ps = [nc.alloc_psum_tensor(f"ps{i}", [128, CO], f32) for i in range(N_ETILE)]

    # single-use DMA-completion sems (see module docstring)
    g8s = [nc.alloc_semaphore(f"s_g8{d}") for d in range(N_GT8_DMAS)]
    z28s = [nc.alloc_semaphore(f"s_z28{g}") for g in range(len(Z28_GROUPS))]
    gts = [nc.alloc_semaphore(f"s_gt{d}") for d in range(len(GT_DMAS))]
    z2s = [nc.alloc_semaphore(f"s_z2{g}") for g in range(len(Z2_GROUPS))]
    s_mm = nc.alloc_semaphore("s_mm")    # chunks consumed by the PE
    s_fin = nc.alloc_semaphore("s_fin")  # v=63 matmuls retired, FIN_ORDER
    s_cpv = nc.alloc_semaphore("s_cpv")  # DVE psum->sbuf copies done
    s_out = nc.alloc_semaphore("s_out")  # out DMAs landed: 16 each

    all_sems = g8s + z28s + gts + z2s + [s_mm, s_fin, s_cpv, s_out]
    nums = [s.num for s in all_sems]
    assert nums == list(range(nums[0], nums[0] + len(nums))), nums
    sem_range = range(nums[0], nums[-1] + 1)
    # cleared at start: only sems without pre-barrier producers (the
    # gt/z2 sems rely on NEFF-load zeroing + the end-of-kernel clear)
    sem_range_start = range(s_mm.num, s_out.num + 1)

    def z28_dma(eng, g):
        a, n = Z28_GROUPS[g]
        eng.dma_start(z2sb8[:, a:a + n, :], z28[:, a:a + n, :]).then_inc(
            z28s[g], 16
        )

    use_midpre = os.environ.get("KOPT_MIDPRE", "1") != "0"
    use_tail = os.environ.get("KOPT_TAIL", "1") != "0"
    use_nowait = os.environ.get("KOPT_NOWAIT", "1") != "0"

    if use_midpre:
        # Relocate the two first-matmul-gating DMAs to just after the
        # framework's const-tile memsets (~5.9 us in, past the runtime's
        # ~2.5 us exec-start DMA-init race window, before the all-engine
        # barrier). gt8 first: the PE's LDWEIGHTS needs it before the
        # rhs, so its flight should land first.
        entry = nc.main_func.blocks[0]
        pre_n = len(entry.instructions)
        nc.scalar.dma_start(gtsb8[:, 0:2, :], gt8[:, 0:2, :]).then_inc(
            g8s[0], 16
        )
        z28_dma(nc.scalar, 0)
        mine = entry.instructions[pre_n:]
        assert len(mine) == 2, len(mine)
        del entry.instructions[pre_n:]
        anchor = 1 + max(
            i for i, ins in enumerate(entry.instructions)
            if type(ins).__name__ == "InstMemset"
        )
        for off, ins in enumerate(mine):
            entry.instructions.insert(anchor + off, ins)

    with nc.Block(name="k", no_gpsimd_drain=True) as blk:

        @blk.sync
        def _(eng):
            eng.sem_clear(sem_range_start)
            if not use_midpre:
                eng.dma_start(gtsb8[:, 0:2, :], gt8[:, 0:2, :]).then_inc(
                    g8s[0], 16
                )
            for i, (a, n) in enumerate(GT8_PLAN[1:]):
                if a >= 12:
                    # light pacing for the last fp8 span
                    eng.wait_ge(s_mm, 2)
                eng.dma_start(
                    gtsb8[:, a:a + n, :], gt8[:, a:a + n, :]
                ).then_inc(g8s[1 + i], 16)
            for d, (a, n) in enumerate(GT_DMAS):
                if a >= 30:
                    # ring-slot reuse: chunk a lands in the slot chunk
                    # a-16 occupied; also covers burst pacing
                    eng.wait_ge(s_mm, max(a + n - NSLOT, a - 12))
                elif a >= 14:
                    # burst pacing through the ramp
                    eng.wait_ge(s_mm, a - 12)
                sl = (a - N8) % NSLOT
                eng.dma_start(
                    gtsb[:, sl:sl + n, :], gt[:, a - N8:a - N8 + n, :]
                ).then_inc(gts[d], 16)
            for k, et in enumerate((0, 2)):
                eng.wait_ge(s_cpv, 2 * (k + 1))
                eng.dma_start(
                    out[:, et:et + 2, :], osb[:, et:et + 2, :]
                ).then_inc(s_out, 16)
            if use_tail:
                # e-tile 7 evacuated by DVE (5th s_cpv inc), shipped here
                eng.wait_ge(s_cpv, 5)
                eng.dma_start(
                    out[:, 7:8, :], osb[:, 7:8, :]
                ).then_inc(s_out, 16)
            if not use_nowait:
                eng.wait_ge(s_out, 16 * 5)
            # leave sems zeroed so a re-execution of the loaded NEFF works
            eng.sem_clear(sem_range)

        @blk.scalar
        def _(eng):
            for g in range(0 if not use_midpre else 1, len(Z28_GROUPS)):
                if Z28_GROUPS[g][0] >= 8:
                    eng.wait_ge(s_mm, 2)
                z28_dma(eng, g)
            for g, (c0, n) in enumerate(Z2_GROUPS):
                if c0 >= 18:
                    # pace Z2 groups by PE consumption (see prior session:
                    # an unpaced burst starved the PE weight prefetch)
                    eng.wait_ge(s_mm, c0 - 12)
                eng.dma_start(
                    z2sb[:, c0 - N8:c0 - N8 + n, :], z2[:, c0 - N8:c0 - N8 + n, :]
                ).then_inc(z2s[g], 16)
            sc_tiles = (4, 5, 6) if use_tail else (4, 5, 6, 7)
            for et in sc_tiles:
                eng.wait_ge(s_fin, FIN_THRESH[et])
                eng.copy(osb[:, et, :], ps[et][:])
                if et == 5:
                    eng.dma_start(
                        out[:, 4:6, :], osb[:, 4:6, :]
                    ).then_inc(s_out, 16)
                elif et >= 6:
                    eng.dma_start(
                        out[:, et:et + 1, :], osb[:, et:et + 1, :]
                    ).then_inc(s_out, 16)

        @blk.tensor
        def _(eng):
            # Build per-step (fp8 pair / bf16 chunk) wait lists, then emit
            # each step's waits just before the PREVIOUS step's last MM:
            # at a boundary the in-order PE queue otherwise serializes
            # [wait][LDWEIGHTS][MM], exposing the ~110-210 ns LDWEIGHTS
            # that mid-chunk hides behind the running MM (observed as
            # 432 ns boundary gaps).
            steps = []
            landed8 = 0
            g8 = 0
            for t in range(NP8):
                w = []
                while landed8 < 2 * t + 2:
                    w.append(z28s[g8])
                    landed8 += Z28_GROUPS[g8][1]
                    g8 += 1
                for i, (a, n) in enumerate(GT8_PLAN):
                    if 2 * t == a:
                        w.append(g8s[i])
                steps.append(("f8", t, w))
            landed = N8
            g = 0
            for v in range(N8, N_VCHUNK):
                w = []
                while v >= landed:
                    w.append(z2s[g])
                    landed += Z2_GROUPS[g][1]
                    g += 1
                d = _GT_IDX[v]
                if v == GT_DMAS[d][0]:
                    w.append(gts[d])
                steps.append(("bf", v, w))
            for si, (kind, idx, waits) in enumerate(steps):
                if si == 0:
                    for s in waits:
                        eng.wait_ge(s, 16)
                nxt = steps[si + 1][2] if si + 1 < len(steps) else []
                for j in range(N_ETILE):
                    et = j
                    if j == N_ETILE - 1:
                        for s in nxt:
                            eng.wait_ge(s, 16)
                    if kind == "f8":
                        t = idx
                        mm = eng.matmul(
                            ps[et][:],
                            lhsT=gtsb8[
                                :, 2 * t:2 * t + 2, et * 128:(et + 1) * 128
                            ],
                            rhs=z2sb8[:, 2 * t:2 * t + 2, :],
                            start=(t == 0),
                            stop=False,
                            perf_mode=DR,
                        )
                        if j == N_ETILE - 1:
                            mm.then_inc(s_mm, 2)
                    else:
                        v = idx
                        last = v == N_VCHUNK - 1
                        sl = (v - N8) % NSLOT
                        mm = eng.matmul(
                            ps[et][:],
                            lhsT=gtsb[:, sl, et * 128:(et + 1) * 128],
                            rhs=z2sb[:, v - N8, :],
                            start=False,
                            stop=last,
                        )
                        if j == N_ETILE - 1 and not last:
                            mm.then_inc(s_mm, 1)
                        if last:
                            mm.then_inc(s_fin, 1)

        @blk.vector
        def _(eng):
            # et7 rides on DVE: after its 4 early-tile copies the DVE is
            # idle while Scalar serially evacuates 4,5,6; the last
            # (critical-path) tile overlaps Scalar's et6 work. Sync ships
            # it on the 5th s_cpv.
            ve_tiles = (0, 1, 2, 3, 7) if use_tail else (0, 1, 2, 3)
            for et in ve_tiles:
                eng.wait_ge(s_fin, FIN_THRESH[et])
                eng.tensor_copy(osb[:, et, :], ps[et][:]).then_inc(s_cpv, 1)

    nc.compile()
    return nc


_cache = {}


def _prep_inputs(x, G, W, b):
    x = np.asarray(x, dtype=np.float32)
    G = np.asarray(G, dtype=np.float32)
    W = np.asarray(W, dtype=np.float32)
    b = np.asarray(b, dtype=np.float32)

    X2 = np.ascontiguousarray(x.reshape(V, KD))                 # [V, (k,d)]
    WM = np.ascontiguousarray(W.transpose(2, 1, 3, 0).reshape(KD, CO))
    bias = b.sum(axis=-1).T.reshape(CO)                          # [(c,o)]
    Z2 = (X2 @ WM + bias[None, :]) * MU                          # [V, CO] scaled

    VS8 = N8 * 128  # 1792 fp8 contraction rows
    # fp8 part, partition-major [128, N8, CO]
    Z28P = np.ascontiguousarray(
        np.clip(Z2[:VS8], -240, 240)
        .astype(F8E4)
        .reshape(N8, 128, CO)
        .transpose(1, 0, 2)
    )
    Z2P = np.ascontiguousarray(
        Z2[VS8:].astype(BF16).reshape(NB, 128, CO).transpose(1, 0, 2)
    )

    GT = G.T * LAM                                               # [V, E] scaled
    GT8 = np.clip(GT[:VS8], -240, 240).astype(F8E4)
    GTB = GT[VS8:].astype(BF16)
    in_maps = []
    for c in range(N_CORES):
        sl = slice(c * EL, (c + 1) * EL)
        GT8P = np.ascontiguousarray(
            GT8[:, sl].reshape(N8, 128, EL).transpose(1, 0, 2)
        )
        GTP = np.ascontiguousarray(
            GTB[:, sl].reshape(NB, 128, EL).transpose(1, 0, 2)
        )
        in_maps.append({"gt8": GT8P, "z28": Z28P, "gt": GTP, "z2": Z2P})
    return in_maps


def _run(x, G, W, b, trace=False, trace_cores=None):
    from concourse.bass_utils import run_bass_kernel_spmd

    if "raw" not in _cache:
        _cache["raw"] = _build_bass_raw()
    nc = _cache["raw"]

    in_maps = _prep_inputs(x, G, W, b)
    kw = {}
    if trace_cores is not None:
        kw["trace_cores"] = trace_cores
    res = run_bass_kernel_spmd(
        nc, in_maps, core_ids=list(range(N_CORES)), trace=trace, **kw,
    )
    # out is [128, 8, 512] fp16 per core (scaled by LAM*MU), row
    # e_loc = et*128 + p
    outs = []
    for c in range(N_CORES):
        o = res.results[c]["out"]
        outs.append(np.ascontiguousarray(o.transpose(1, 0, 2)).reshape(EL, CO))
    out = np.concatenate(outs, axis=0).astype(np.float32) * (1.0 / (LAM * MU))
    out = out.reshape(E, C, O)
    return out, res


def kernel(x, G, W, b):
    out, _ = _run(x, G, W, b, trace=False)
    return out


# revision 17
# speedup vs baseline: 1.1348x; 1.0045x over previous
"""Trainium2 Bass kernel for nn_BlockLTN (gnn_message_passing).

Math:
    z[o,v,c] = sum_{k,d} x[v,k,d] * W[o,d,k,c] + sum_d b[o,c,d]
    out[e,c,o] = sum_v G[e,v] * z[o,v,c]

Folded:  out[e, c*8+o] = G[e,:] @ Z2[:, c*8+o]
  where  Z2[v, c*8+o] = (x.reshape(V,KD) @ W.transpose(2,1,3,0).reshape(KD,CO))[v, c*8+o]
                        + b.sum(-1).T.reshape(CO)[c*8+o]

The dominant work is the [E,V] @ [V, CO] GEMM over the 256 MB boundary
operator G (68.7 GFLOP); Z2 is a 4.3 GFLOP preprocessing folded on host.
Sharding (per hint): G and out row-wise over E across 8 cores (data
parallel over out-simplices); Z2 replicated; no collectives.

MIXED PRECISION: the first N8=14 v-chunks (1792 of 8192 contraction
rows, 21.9%) run as 7 fp8-e4m3 DoubleRow matmuls per e-tile (2 weights
per PE cell, 2 MACs/cycle); the remaining 50 chunks run bf16 at the
1 column/cycle silicon limit (216 ns per 128x128x512 MM). Error budget:
e4m3 quantization of both operands costs ~3.7% rel err at full
coverage; at 21.9% coverage the measured host-sim error is 1.76e-2 vs
the 2e-2 gate (bf16-only was 2.35e-3). Scaling: G ships x128 and Z2 x8
(exact powers of two, applied to BOTH precisions so the mixed PSUM
accumulation is uniform; G*128 lifts sigma to 2.56 so values clear the
e4m3 min-normal 0.015625 -- unscaled, 56% of G is subnormal garbage);
the host divides the fp16 output by 1024 (max |out*1024| ~ 27k < 65504).
fp8 pairs run FIRST: the HAM-cold window (~3.4 us at 1.2 GHz from
kernel start) covers work at 2 MACs/cycle, and the delivery-bound ramp
needs half the DMA bytes.

SYNCHRONIZATION (inherited, load-bearing): a DMA's `.then_inc(sem, 16)`
is performed as SIXTEEN independent +1 increments, one per SDMA engine.
With several DMAs in flight on one semaphore, a later DMA's increments
can satisfy an earlier DMA's cumulative threshold while one lagging
SDMA engine still owes its 8 partitions -> rank-8 stale-data corruption
(observed under NTFF profiling skew). Therefore every DMA a consumer
waits on gets its OWN single-use semaphore (threshold 16 == fully
landed).

Tail (v=63, bf16, unchanged order 0..7 -- a reordered tail shipped
garbage intermittently in a prior session): Vector evacuates e-tiles
0-3 and 7 (PSUM->SBUF fp16), Scalar 4-6; Sync ships 0:2, 2:4 and 7,
Scalar ships 4:6, 6. The final all-DMAs-landed wait (s_out) is DROPPED:
the runtime's end-of-NEFF epilogue (all-engine barrier + ~6.9 us of
per-engine semaphore clears + final barrier, appended by the runtime
after this program) runs before the host can observe completion, and
the last out-DMA receipt (~3.2 us) lands well inside it. s_out residue
from late receipts is wiped by the start-of-block clear on the next
execution.

Preamble: the profiler's exec-time clock starts at the framework's
const-tile MEMSETs (~6.05 us into the runtime preamble); the two DMAs
gating the first matmul (z28 chunks 0:2 and gt8 chunks 0:2 cols 0:512)
are relocated into the preamble right after those MEMSETs on the
Scalar queue -- past the runtime's ~2.5 us exec-start DMA-init race
window that corrupted a start-of-preamble variant, but ~1.2 us before
the all-engine barrier releases.
"""

import os

import numpy as np
import ml_dtypes

V = 8192
E = 8192
K = 64
C = 64
D = 8
O = 8
KD = K * D    # 512
CO = C * O    # 512
N_CORES = 8
EL = E // N_CORES  # 1024 out-rows per core
N_VCHUNK = V // 128  # 64
N_ETILE = EL // 128  # 8

N8 = 16               # fp8 chunks (v 0..15) -> 8 DoubleRow pairs
NP8 = N8 // 2
NB = N_VCHUNK - N8    # 48 bf16 chunks (v 16..63)
NSLOT = 16            # bf16 gt ring depth
LAM = 128.0           # G scale (2**7)
MU = 8.0              # Z2 scale (2**3)

BF16 = ml_dtypes.bfloat16
F8E4 = ml_dtypes.float8_e4m3  # TRN fp8_exp4: max +-240

# fp8 GT dma plan, in fp8-chunk spans. dma 0 (chunks 0:2, pre-issued
# mid-preamble on Scalar) ships WHOLE: a column split halves the
# per-partition contiguous run to 512 B and the descriptor overhead
# stretched the first flight 3.0 -> 4.3 us; unsplit it merges to 2 KB.
GT8_PLAN = [(0, 2), (2, 2), (4, 4), (8, 4), (12, 4)]
N_GT8_DMAS = len(GT8_PLAN)
Z28_GROUPS = [(0, 2), (2, 2), (4, 4), (8, 4), (12, 4)]  # first pre-issued

# bf16 GT dmas (chunks 16..63) through the 16-slot ring, slot (a-16)%16.
# Spans are chosen so no dma wraps the ring boundary.
GT_DMAS = [(16, 2), (18, 2)] + [(20 + 4 * t, 4) for t in range(11)]
assert sum(n for _, n in GT_DMAS) == NB
for _a, _n in GT_DMAS:
    assert (_a - N8) % NSLOT + _n <= NSLOT, (_a, _n)
_GT_IDX = {}
for _d, (_a, _n) in enumerate(GT_DMAS):
    for _c in range(_a, _a + _n):
        _GT_IDX[_c] = _d

Z2_GROUPS = [(16, 2), (18, 2)] + [(20 + 4 * t, 4) for t in range(11)]
assert sum(n for _, n in Z2_GROUPS) == NB

# v=63 e-tile emission order; each final MM bumps s_fin.
FIN_ORDER = list(range(N_ETILE))
FIN_THRESH = {et: et + 1 for et in FIN_ORDER}


def _build_bass_raw():
    import concourse.mybir as mybir
    from concourse import bacc

    f32 = mybir.dt.float32
    fp16 = mybir.dt.float16
    bf16 = mybir.dt.bfloat16
    f8e4 = mybir.dt.float8e4
    DR = mybir.MatmulPerfMode.DoubleRow

    nc = bacc.Bacc("TRN2", target_bir_lowering=False)

    # partition-major layouts prepared on host
    gt8 = nc.dram_tensor("gt8", (128, N8, EL), f8e4, kind="ExternalInput")
    z28 = nc.dram_tensor("z28", (128, N8, CO), f8e4, kind="ExternalInput")
    gt = nc.dram_tensor("gt", (128, NB, EL), bf16, kind="ExternalInput")
    z2 = nc.dram_tensor("z2", (128, NB, CO), bf16, kind="ExternalInput")
    out = nc.dram_tensor("out", (128, N_ETILE, CO), fp16, kind="ExternalOutput")

    gtsb8 = nc.alloc_sbuf_tensor("gtsb8", [128, N8, EL], f8e4)
    z2sb8 = nc.alloc_sbuf_tensor("z2sb8", [128, N8, CO], f8e4)
    gtsb = nc.alloc_sbuf_tensor("gtsb", [128, NSLOT, EL], bf16)
    z2sb = nc.alloc_sbuf_tensor("z2sb", [128, NB, CO], bf16)
    osb = nc.alloc_sbuf_tensor("osb", [128, N_ETILE, CO], fp16)
    ps = [nc.alloc_psum_tensor(f"ps{i}", [128, CO], f32) for i in range(N_ETILE)]

    # single-use DMA-completion sems (see module docstring)
    g8s = [nc.alloc_semaphore(f"s_g8{d}") for d in range(N_GT8_DMAS)]
    z28s = [nc.alloc_semaphore(f"s_z28{g}") for g in range(len(Z28_GROUPS))]
    gts = [nc.alloc_semaphore(f"s_gt{d}") for d in range(len(GT_DMAS))]
    z2s = [nc.alloc_semaphore(f"s_z2{g}") for g in range(len(Z2_GROUPS))]
    s_mm = nc.alloc_semaphore("s_mm")    # chunks consumed by the PE
    s_fin = nc.alloc_semaphore("s_fin")  # v=63 matmuls retired, FIN_ORDER
    s_cpv = nc.alloc_semaphore("s_cpv")  # DVE psum->sbuf copies done
    s_out = nc.alloc_semaphore("s_out")  # out DMAs landed: 16 each

    all_sems = g8s + z28s + gts + z2s + [s_mm, s_fin, s_cpv, s_out]
    nums = [s.num for s in all_sems]
    assert nums == list(range(nums[0], nums[0] + len(nums))), nums
    sem_range = range(nums[0], nums[-1] + 1)
    # cleared at start: only sems without pre-barrier producers (the
    # gt/z2 sems rely on NEFF-load zeroing + the end-of-kernel clear)
    sem_range_start = range(s_mm.num, s_out.num + 1)

    def z28_dma(eng, g):
        a, n = Z28_GROUPS[g]
        eng.dma_start(z2sb8[:, a:a + n, :], z28[:, a:a + n, :]).then_inc(
            z28s[g], 16
        )

    use_midpre = os.environ.get("KOPT_MIDPRE", "1") != "0"
    use_tail = os.environ.get("KOPT_TAIL", "1") != "0"
    use_nowait = os.environ.get("KOPT_NOWAIT", "1") != "0"

    if use_midpre:
        # Relocate the two first-matmul-gating DMAs to just after the
        # framework's const-tile memsets (~5.9 us in, past the runtime's
        # ~2.5 us exec-start DMA-init race window, before the all-engine
        # barrier). gt8 first: the PE's LDWEIGHTS needs it before the
        # rhs, so its flight should land first.
        entry = nc.main_func.blocks[0]
        pre_n = len(entry.instructions)
        nc.scalar.dma_start(gtsb8[:, 0:2, :], gt8[:, 0:2, :]).then_inc(
            g8s[0], 16
        )
        z28_dma(nc.scalar, 0)
        mine = entry.instructions[pre_n:]
        assert len(mine) == 2, len(mine)
        del entry.instructions[pre_n:]
        anchor = 1 + max(
            i for i, ins in enumerate(entry.instructions)
            if type(ins).__name__ == "InstMemset"
        )
        for off, ins in enumerate(mine):
            entry.instructions.insert(anchor + off, ins)

    with nc.Block(name="k", no_gpsimd_drain=True) as blk:

        @blk.sync
        def _(eng):
            eng.sem_clear(sem_range_start)
            if not use_midpre:
                eng.dma_start(gtsb8[:, 0:2, :], gt8[:, 0:2, :]).then_inc(
                    g8s[0], 16
                )
            for i, (a, n) in enumerate(GT8_PLAN[1:]):
                if a >= 12:
                    # light pacing for the last fp8 span
                    eng.wait_ge(s_mm, 2)
                eng.dma_start(
                    gtsb8[:, a:a + n, :], gt8[:, a:a + n, :]
                ).then_inc(g8s[1 + i], 16)
            for d, (a, n) in enumerate(GT_DMAS):
                if a >= 30:
                    # ring-slot reuse: chunk a lands in the slot chunk
                    # a-16 occupied; also covers burst pacing
                    eng.wait_ge(s_mm, max(a + n - NSLOT, a - 12))
                elif a >= 14:
                    # burst pacing through the ramp
                    eng.wait_ge(s_mm, a - 12)
                sl = (a - N8) % NSLOT
                eng.dma_start(
                    gtsb[:, sl:sl + n, :], gt[:, a - N8:a - N8 + n, :]
                ).then_inc(gts[d], 16)
            for k, et in enumerate((0, 2)):
                eng.wait_ge(s_cpv, 2 * (k + 1))
                eng.dma_start(
                    out[:, et:et + 2, :], osb[:, et:et + 2, :]
                ).then_inc(s_out, 16)
            if use_tail:
                # e-tile 7 evacuated by DVE (5th s_cpv inc), shipped here
                eng.wait_ge(s_cpv, 5)
                eng.dma_start(
                    out[:, 7:8, :], osb[:, 7:8, :]
                ).then_inc(s_out, 16)
            if not use_nowait:
                eng.wait_ge(s_out, 16 * 5)
            # leave sems zeroed so a re-execution of the loaded NEFF works
            eng.sem_clear(sem_range)

        @blk.scalar
        def _(eng):
            for g in range(0 if not use_midpre else 1, len(Z28_GROUPS)):
                if Z28_GROUPS[g][0] >= 8:
                    eng.wait_ge(s_mm, 2)
                z28_dma(eng, g)
            for g, (c0, n) in enumerate(Z2_GROUPS):
                if c0 >= 18:
                    # pace Z2 groups by PE consumption (see prior session:
                    # an unpaced burst starved the PE weight prefetch)
                    eng.wait_ge(s_mm, c0 - 12)
                eng.dma_start(
                    z2sb[:, c0 - N8:c0 - N8 + n, :], z2[:, c0 - N8:c0 - N8 + n, :]
                ).then_inc(z2s[g], 16)
            sc_tiles = (4, 5, 6) if use_tail else (4, 5, 6, 7)
            for et in sc_tiles:
                eng.wait_ge(s_fin, FIN_THRESH[et])
                eng.copy(osb[:, et, :], ps[et][:])
                if et == 5:
                    eng.dma_start(
                        out[:, 4:6, :], osb[:, 4:6, :]
                    ).then_inc(s_out, 16)
                elif et >= 6:
                    eng.dma_start(
                        out[:, et:et + 1, :], osb[:, et:et + 1, :]
                    ).then_inc(s_out, 16)

        @blk.tensor
        def _(eng):
            # HAM warm-up: the PE exits the preamble barrier ~2.1 us before
            # the first input chunks land; garbage matmuls (into ps[0],
            # reset by the real start=True) keep the PE-busy activity
            # window saturated from barrier exit so the HAM un-throttle
            # (K=4/8 -> 8/8) fires ~1-2 us earlier into the real stream.
            # Data raced with the in-flight first DMAs is discarded.
            n_warm = int(os.environ.get("KOPT_WARM", "4"))
            for i in range(n_warm):
                eng.matmul(
                    ps[0][:],
                    lhsT=gtsb8[:, 0:2, 0:128],
                    rhs=z2sb8[:, 0:2, :],
                    start=(i == 0),
                    stop=(i == n_warm - 1),
                    perf_mode=DR,
                )
            # Build per-step (fp8 pair / bf16 chunk) wait lists, then emit
            # each step's waits just before the PREVIOUS step's last MM:
            # at a boundary the in-order PE queue otherwise serializes
            # [wait][LDWEIGHTS][MM], exposing the ~110-210 ns LDWEIGHTS
            # that mid-chunk hides behind the running MM (observed as
            # 432 ns boundary gaps).
            steps = []
            landed8 = 0
            g8 = 0
            for t in range(NP8):
                w = []
                while landed8 < 2 * t + 2:
                    w.append(z28s[g8])
                    landed8 += Z28_GROUPS[g8][1]
                    g8 += 1
                for i, (a, n) in enumerate(GT8_PLAN):
                    if 2 * t == a:
                        w.append(g8s[i])
                steps.append(("f8", t, w))
            landed = N8
            g = 0
            for v in range(N8, N_VCHUNK):
                w = []
                while v >= landed:
                    w.append(z2s[g])
                    landed += Z2_GROUPS[g][1]
                    g += 1
                d = _GT_IDX[v]
                if v == GT_DMAS[d][0]:
                    w.append(gts[d])
                steps.append(("bf", v, w))
            for si, (kind, idx, waits) in enumerate(steps):
                if si == 0:
                    for s in waits:
                        eng.wait_ge(s, 16)
                nxt = steps[si + 1][2] if si + 1 < len(steps) else []
                for j in range(N_ETILE):
                    et = j
                    if j == N_ETILE - 1:
                        for s in nxt:
                            eng.wait_ge(s, 16)
                    if kind == "f8":
                        t = idx
                        mm = eng.matmul(
                            ps[et][:],
                            lhsT=gtsb8[
                                :, 2 * t:2 * t + 2, et * 128:(et + 1) * 128
                            ],
                            rhs=z2sb8[:, 2 * t:2 * t + 2, :],
                            start=(t == 0),
                            stop=False,
                            perf_mode=DR,
                        )
                        if j == N_ETILE - 1:
                            mm.then_inc(s_mm, 2)
                    else:
                        v = idx
                        last = v == N_VCHUNK - 1
                        sl = (v - N8) % NSLOT
                        mm = eng.matmul(
                            ps[et][:],
                            lhsT=gtsb[:, sl, et * 128:(et + 1) * 128],
                            rhs=z2sb[:, v - N8, :],
                            start=False,
                            stop=last,
                        )
                        if j == N_ETILE - 1 and not last:
                            mm.then_inc(s_mm, 1)
                        if last:
                            mm.then_inc(s_fin, 1)

        @blk.vector
        def _(eng):
            # et7 rides on DVE: after its 4 early-tile copies the DVE is
            # idle while Scalar serially evacuates 4,5,6; the last
            # (critical-path) tile overlaps Scalar's et6 work. Sync ships
            # it on the 5th s_cpv.
            ve_tiles = (0, 1, 2, 3, 7) if use_tail else (0, 1, 2, 3)
            for et in ve_tiles:
                eng.wait_ge(s_fin, FIN_THRESH[et])
                eng.tensor_copy(osb[:, et, :], ps[et][:]).then_inc(s_cpv, 1)

    nc.compile()
    return nc


_cache = {}


def _prep_inputs(x, G, W, b):
    x = np.asarray(x, dtype=np.float32)
    G = np.asarray(G, dtype=np.float32)
    W = np.asarray(W, dtype=np.float32)
    b = np.asarray(b, dtype=np.float32)

    X2 = np.ascontiguousarray(x.reshape(V, KD))                 # [V, (k,d)]
    WM = np.ascontiguousarray(W.transpose(2, 1, 3, 0).reshape(KD, CO))
    bias = b.sum(axis=-1).T.reshape(CO)                          # [(c,o)]
    Z2 = (X2 @ WM + bias[None, :]) * MU                          # [V, CO] scaled

    VS8 = N8 * 128  # 1792 fp8 contraction rows
    # fp8 part, partition-major [128, N8, CO]
    Z28P = np.ascontiguousarray(
        np.clip(Z2[:VS8], -240, 240)
        .astype(F8E4)
        .reshape(N8, 128, CO)
        .transpose(1, 0, 2)
    )
    Z2P = np.ascontiguousarray(
        Z2[VS8:].astype(BF16).reshape(NB, 128, CO).transpose(1, 0, 2)
    )

    GT = G.T * LAM                                               # [V, E] scaled
    GT8 = np.clip(GT[:VS8], -240, 240).astype(F8E4)
    GTB = GT[VS8:].astype(BF16)
    in_maps = []
    for c in range(N_CORES):
        sl = slice(c * EL, (c + 1) * EL)
        GT8P = np.ascontiguousarray(
            GT8[:, sl].reshape(N8, 128, EL).transpose(1, 0, 2)
        )
        GTP = np.ascontiguousarray(
            GTB[:, sl].reshape(NB, 128, EL).transpose(1, 0, 2)
        )
        in_maps.append({"gt8": GT8P, "z28": Z28P, "gt": GTP, "z2": Z2P})
    return in_maps


def _run(x, G, W, b, trace=False, trace_cores=None):
    from concourse.bass_utils import run_bass_kernel_spmd

    if "raw" not in _cache:
        _cache["raw"] = _build_bass_raw()
    nc = _cache["raw"]

    in_maps = _prep_inputs(x, G, W, b)
    kw = {}
    if trace_cores is not None:
        kw["trace_cores"] = trace_cores
    res = run_bass_kernel_spmd(
        nc, in_maps, core_ids=list(range(N_CORES)), trace=trace, **kw,
    )
    # out is [128, 8, 512] fp16 per core (scaled by LAM*MU), row
    # e_loc = et*128 + p
    outs = []
    for c in range(N_CORES):
        o = res.results[c]["out"]
        outs.append(np.ascontiguousarray(o.transpose(1, 0, 2)).reshape(EL, CO))
    out = np.concatenate(outs, axis=0).astype(np.float32) * (1.0 / (LAM * MU))
    out = out.reshape(E, C, O)
    return out, res


def kernel(x, G, W, b):
    out, _ = _run(x, G, W, b, trace=False)
    return out


# revision 19
# speedup vs baseline: 1.1428x; 1.0071x over previous
"""Trainium2 Bass kernel for nn_BlockLTN (gnn_message_passing).

Math:
    z[o,v,c] = sum_{k,d} x[v,k,d] * W[o,d,k,c] + sum_d b[o,c,d]
    out[e,c,o] = sum_v G[e,v] * z[o,v,c]

Folded:  out[e, c*8+o] = G[e,:] @ Z2[:, c*8+o]
  where  Z2[v, c*8+o] = (x.reshape(V,KD) @ W.transpose(2,1,3,0).reshape(KD,CO))[v, c*8+o]
                        + b.sum(-1).T.reshape(CO)[c*8+o]

The dominant work is the [E,V] @ [V, CO] GEMM over the 256 MB boundary
operator G (68.7 GFLOP); Z2 is a 4.3 GFLOP preprocessing folded on host.
Sharding (per hint): G and out row-wise over E across 8 cores (data
parallel over out-simplices); Z2 replicated; no collectives.

MIXED PRECISION: the first N8=14 v-chunks (1792 of 8192 contraction
rows, 21.9%) run as 7 fp8-e4m3 DoubleRow matmuls per e-tile (2 weights
per PE cell, 2 MACs/cycle); the remaining 50 chunks run bf16 at the
1 column/cycle silicon limit (216 ns per 128x128x512 MM). Error budget:
e4m3 quantization of both operands costs ~3.7% rel err at full
coverage; at 21.9% coverage the measured host-sim error is 1.76e-2 vs
the 2e-2 gate (bf16-only was 2.35e-3). Scaling: G ships x128 and Z2 x8
(exact powers of two, applied to BOTH precisions so the mixed PSUM
accumulation is uniform; G*128 lifts sigma to 2.56 so values clear the
e4m3 min-normal 0.015625 -- unscaled, 56% of G is subnormal garbage);
the host divides the fp16 output by 1024 (max |out*1024| ~ 27k < 65504).
fp8 pairs run FIRST: the HAM-cold window (~3.4 us at 1.2 GHz from
kernel start) covers work at 2 MACs/cycle, and the delivery-bound ramp
needs half the DMA bytes.

SYNCHRONIZATION (inherited, load-bearing): a DMA's `.then_inc(sem, 16)`
is performed as SIXTEEN independent +1 increments, one per SDMA engine.
With several DMAs in flight on one semaphore, a later DMA's increments
can satisfy an earlier DMA's cumulative threshold while one lagging
SDMA engine still owes its 8 partitions -> rank-8 stale-data corruption
(observed under NTFF profiling skew). Therefore every DMA a consumer
waits on gets its OWN single-use semaphore (threshold 16 == fully
landed).

Tail (v=63, bf16, unchanged order 0..7 -- a reordered tail shipped
garbage intermittently in a prior session): Vector evacuates e-tiles
0-3 and 7 (PSUM->SBUF fp16), Scalar 4-6; Sync ships 0:2, 2:4 and 7,
Scalar ships 4:6, 6. The final all-DMAs-landed wait (s_out) is DROPPED:
the runtime's end-of-NEFF epilogue (all-engine barrier + ~6.9 us of
per-engine semaphore clears + final barrier, appended by the runtime
after this program) runs before the host can observe completion, and
the last out-DMA receipt (~3.2 us) lands well inside it. s_out residue
from late receipts is wiped by the start-of-block clear on the next
execution.

Preamble: the profiler's exec-time clock starts at the framework's
const-tile MEMSETs (~6.05 us into the runtime preamble); the two DMAs
gating the first matmul (z28 chunks 0:2 and gt8 chunks 0:2 cols 0:512)
are relocated into the preamble right after those MEMSETs on the
Scalar queue -- past the runtime's ~2.5 us exec-start DMA-init race
window that corrupted a start-of-preamble variant, but ~1.2 us before
the all-engine barrier releases.
"""

import os

import numpy as np
import ml_dtypes

V = 8192
E = 8192
K = 64
C = 64
D = 8
O = 8
KD = K * D    # 512
CO = C * O    # 512
N_CORES = 8
EL = E // N_CORES  # 1024 out-rows per core
N_VCHUNK = V // 128  # 64
N_ETILE = EL // 128  # 8

N8 = 16               # fp8 chunks (v 0..15) -> 8 DoubleRow pairs
NP8 = N8 // 2
NB = N_VCHUNK - N8    # 48 bf16 chunks (v 16..63)
NSLOT = 16            # bf16 gt ring depth
LAM = 128.0           # G scale (2**7)
MU = 8.0              # Z2 scale (2**3)

BF16 = ml_dtypes.bfloat16
F8E4 = ml_dtypes.float8_e4m3  # TRN fp8_exp4: max +-240

# fp8 GT dma plan, in fp8-chunk spans. dma 0 (chunks 0:2, pre-issued
# mid-preamble on Scalar) ships WHOLE: a column split halves the
# per-partition contiguous run to 512 B and the descriptor overhead
# stretched the first flight 3.0 -> 4.3 us; unsplit it merges to 2 KB.
GT8_PLAN = [(0, 2), (2, 2), (4, 4), (8, 4), (12, 4)]
N_GT8_DMAS = len(GT8_PLAN)
Z28_GROUPS = [(0, 2), (2, 2), (4, 4), (8, 4), (12, 4)]  # first pre-issued

# bf16 GT dmas (chunks 16..63) through the 16-slot ring, slot (a-16)%16.
# Spans are chosen so no dma wraps the ring boundary.
GT_DMAS = [(16, 2), (18, 2)] + [(20 + 4 * t, 4) for t in range(11)]
assert sum(n for _, n in GT_DMAS) == NB
for _a, _n in GT_DMAS:
    assert (_a - N8) % NSLOT + _n <= NSLOT, (_a, _n)
_GT_IDX = {}
for _d, (_a, _n) in enumerate(GT_DMAS):
    for _c in range(_a, _a + _n):
        _GT_IDX[_c] = _d

Z2_GROUPS = [(16, 2), (18, 2)] + [(20 + 4 * t, 4) for t in range(11)]
assert sum(n for _, n in Z2_GROUPS) == NB

# v=63 e-tile emission order; each final MM bumps s_fin.
FIN_ORDER = list(range(N_ETILE))
FIN_THRESH = {et: et + 1 for et in FIN_ORDER}


def _build_bass_raw():
    import concourse.mybir as mybir
    from concourse import bacc

    f32 = mybir.dt.float32
    fp16 = mybir.dt.float16
    bf16 = mybir.dt.bfloat16
    f8e4 = mybir.dt.float8e4
    DR = mybir.MatmulPerfMode.DoubleRow

    nc = bacc.Bacc("TRN2", target_bir_lowering=False)

    # partition-major layouts prepared on host
    gt8 = nc.dram_tensor("gt8", (128, N8, EL), f8e4, kind="ExternalInput")
    z28 = nc.dram_tensor("z28", (128, N8, CO), f8e4, kind="ExternalInput")
    gt = nc.dram_tensor("gt", (128, NB, EL), bf16, kind="ExternalInput")
    z2 = nc.dram_tensor("z2", (128, NB, CO), bf16, kind="ExternalInput")
    out = nc.dram_tensor("out", (128, N_ETILE, CO), fp16, kind="ExternalOutput")

    gtsb8 = nc.alloc_sbuf_tensor("gtsb8", [128, N8, EL], f8e4)
    z2sb8 = nc.alloc_sbuf_tensor("z2sb8", [128, N8, CO], f8e4)
    gtsb = nc.alloc_sbuf_tensor("gtsb", [128, NSLOT, EL], bf16)
    z2sb = nc.alloc_sbuf_tensor("z2sb", [128, NB, CO], bf16)
    osb = nc.alloc_sbuf_tensor("osb", [128, N_ETILE, CO], fp16)
    ps = [nc.alloc_psum_tensor(f"ps{i}", [128, CO], f32) for i in range(N_ETILE)]

    # single-use DMA-completion sems (see module docstring)
    g8s = [nc.alloc_semaphore(f"s_g8{d}") for d in range(N_GT8_DMAS)]
    z28s = [nc.alloc_semaphore(f"s_z28{g}") for g in range(len(Z28_GROUPS))]
    gts = [nc.alloc_semaphore(f"s_gt{d}") for d in range(len(GT_DMAS))]
    z2s = [nc.alloc_semaphore(f"s_z2{g}") for g in range(len(Z2_GROUPS))]
    s_mm = nc.alloc_semaphore("s_mm")    # chunks consumed by the PE
    s_fin = nc.alloc_semaphore("s_fin")  # v=63 matmuls retired, FIN_ORDER
    s_cpv = nc.alloc_semaphore("s_cpv")  # DVE psum->sbuf copies done
    s_out = nc.alloc_semaphore("s_out")  # out DMAs landed: 16 each

    all_sems = g8s + z28s + gts + z2s + [s_mm, s_fin, s_cpv, s_out]
    nums = [s.num for s in all_sems]
    assert nums == list(range(nums[0], nums[0] + len(nums))), nums
    sem_range = range(nums[0], nums[-1] + 1)
    # cleared at start: only sems without pre-barrier producers (the
    # gt/z2 sems rely on NEFF-load zeroing + the end-of-kernel clear)
    sem_range_start = range(s_mm.num, s_out.num + 1)

    def z28_dma(eng, g):
        a, n = Z28_GROUPS[g]
        eng.dma_start(z2sb8[:, a:a + n, :], z28[:, a:a + n, :]).then_inc(
            z28s[g], 16
        )

    use_midpre = os.environ.get("KOPT_MIDPRE", "1") != "0"
    use_tail = os.environ.get("KOPT_TAIL", "1") != "0"
    use_nowait = os.environ.get("KOPT_NOWAIT", "1") != "0"

    if use_midpre:
        # Relocate the two first-matmul-gating DMAs to just after the
        # framework's const-tile memsets (~5.9 us in, past the runtime's
        # ~2.5 us exec-start DMA-init race window, before the all-engine
        # barrier). gt8 first: the PE's LDWEIGHTS needs it before the
        # rhs, so its flight should land first.
        entry = nc.main_func.blocks[0]
        pre_n = len(entry.instructions)
        nc.scalar.dma_start(gtsb8[:, 0:2, :], gt8[:, 0:2, :]).then_inc(
            g8s[0], 16
        )
        z28_dma(nc.scalar, 0)
        mine = entry.instructions[pre_n:]
        assert len(mine) == 2, len(mine)
        del entry.instructions[pre_n:]
        anchor = 1 + max(
            i for i, ins in enumerate(entry.instructions)
            if type(ins).__name__ == "InstMemset"
        )
        for off, ins in enumerate(mine):
            entry.instructions.insert(anchor + off, ins)

    with nc.Block(name="k", no_gpsimd_drain=True) as blk:

        @blk.sync
        def _(eng):
            eng.sem_clear(sem_range_start)
            if not use_midpre:
                eng.dma_start(gtsb8[:, 0:2, :], gt8[:, 0:2, :]).then_inc(
                    g8s[0], 16
                )
            for i, (a, n) in enumerate(GT8_PLAN[1:]):
                # fp8 spans ship unpaced: the whole fp8 working set is
                # needed within the first ~24 us and ramp-window DMA
                # latency is 4-7 us under 8-core congestion
                eng.dma_start(
                    gtsb8[:, a:a + n, :], gt8[:, a:a + n, :]
                ).then_inc(g8s[1 + i], 16)
            for d, (a, n) in enumerate(GT_DMAS):
                # bf16 spans are paced ~8 chunks ahead of consumption so
                # they do not contend with the fp8 ramp deliveries; the
                # max() term covers ring-slot reuse (chunk a lands in the
                # slot chunk a-16 occupied)
                eng.wait_ge(s_mm, max(a + n - NSLOT, a - 8))
                sl = (a - N8) % NSLOT
                eng.dma_start(
                    gtsb[:, sl:sl + n, :], gt[:, a - N8:a - N8 + n, :]
                ).then_inc(gts[d], 16)
            for k, et in enumerate((0, 2)):
                eng.wait_ge(s_cpv, 2 * (k + 1))
                eng.dma_start(
                    out[:, et:et + 2, :], osb[:, et:et + 2, :]
                ).then_inc(s_out, 16)
            if use_tail:
                # e-tile 7 evacuated by DVE (5th s_cpv inc), shipped here
                eng.wait_ge(s_cpv, 5)
                eng.dma_start(
                    out[:, 7:8, :], osb[:, 7:8, :]
                ).then_inc(s_out, 16)
            if not use_nowait:
                eng.wait_ge(s_out, 16 * 5)
            # leave sems zeroed so a re-execution of the loaded NEFF works
            eng.sem_clear(sem_range)

        @blk.scalar
        def _(eng):
            for g in range(0 if not use_midpre else 1, len(Z28_GROUPS)):
                z28_dma(eng, g)
            for g, (c0, n) in enumerate(Z2_GROUPS):
                # pace ~8 chunks ahead of consumption (see GT_DMAS note)
                eng.wait_ge(s_mm, c0 - 8)
                eng.dma_start(
                    z2sb[:, c0 - N8:c0 - N8 + n, :], z2[:, c0 - N8:c0 - N8 + n, :]
                ).then_inc(z2s[g], 16)
            sc_tiles = (4, 5, 6) if use_tail else (4, 5, 6, 7)
            for et in sc_tiles:
                eng.wait_ge(s_fin, FIN_THRESH[et])
                eng.copy(osb[:, et, :], ps[et][:])
                if et == 5:
                    eng.dma_start(
                        out[:, 4:6, :], osb[:, 4:6, :]
                    ).then_inc(s_out, 16)
                elif et >= 6:
                    eng.dma_start(
                        out[:, et:et + 1, :], osb[:, et:et + 1, :]
                    ).then_inc(s_out, 16)

        @blk.tensor
        def _(eng):
            # HAM warm-up: the PE exits the preamble barrier ~2.1 us before
            # the first input chunks land; garbage matmuls (into ps[0],
            # reset by the real start=True) keep the PE-busy activity
            # window saturated from barrier exit so the HAM un-throttle
            # (K=4/8 -> 8/8) fires ~1-2 us earlier into the real stream.
            # Data raced with the in-flight first DMAs is discarded.
            n_warm = int(os.environ.get("KOPT_WARM", "4"))
            for i in range(n_warm):
                eng.matmul(
                    ps[0][:],
                    lhsT=gtsb8[:, 0:2, 0:128],
                    rhs=z2sb8[:, 0:2, :],
                    start=(i == 0),
                    stop=(i == n_warm - 1),
                    perf_mode=DR,
                )
            # Build per-step (fp8 pair / bf16 chunk) wait lists, then emit
            # each step's waits just before the PREVIOUS step's last MM:
            # at a boundary the in-order PE queue otherwise serializes
            # [wait][LDWEIGHTS][MM], exposing the ~110-210 ns LDWEIGHTS
            # that mid-chunk hides behind the running MM (observed as
            # 432 ns boundary gaps).
            steps = []
            landed8 = 0
            g8 = 0
            for t in range(NP8):
                w = []
                while landed8 < 2 * t + 2:
                    w.append(z28s[g8])
                    landed8 += Z28_GROUPS[g8][1]
                    g8 += 1
                for i, (a, n) in enumerate(GT8_PLAN):
                    if 2 * t == a:
                        w.append(g8s[i])
                steps.append(("f8", t, w))
            landed = N8
            g = 0
            for v in range(N8, N_VCHUNK):
                w = []
                while v >= landed:
                    w.append(z2s[g])
                    landed += Z2_GROUPS[g][1]
                    g += 1
                d = _GT_IDX[v]
                if v == GT_DMAS[d][0]:
                    w.append(gts[d])
                steps.append(("bf", v, w))
            for si, (kind, idx, waits) in enumerate(steps):
                if si == 0:
                    for s in waits:
                        eng.wait_ge(s, 16)
                nxt = steps[si + 1][2] if si + 1 < len(steps) else []
                for j in range(N_ETILE):
                    et = j
                    if j == N_ETILE - 1:
                        for s in nxt:
                            eng.wait_ge(s, 16)
                    if kind == "f8":
                        t = idx
                        mm = eng.matmul(
                            ps[et][:],
                            lhsT=gtsb8[
                                :, 2 * t:2 * t + 2, et * 128:(et + 1) * 128
                            ],
                            rhs=z2sb8[:, 2 * t:2 * t + 2, :],
                            start=(t == 0),
                            stop=False,
                            perf_mode=DR,
                        )
                        if j == N_ETILE - 1:
                            mm.then_inc(s_mm, 2)
                    else:
                        v = idx
                        last = v == N_VCHUNK - 1
                        sl = (v - N8) % NSLOT
                        mm = eng.matmul(
                            ps[et][:],
                            lhsT=gtsb[:, sl, et * 128:(et + 1) * 128],
                            rhs=z2sb[:, v - N8, :],
                            start=False,
                            stop=last,
                        )
                        if j == N_ETILE - 1 and not last:
                            mm.then_inc(s_mm, 1)
                        if last:
                            mm.then_inc(s_fin, 1)

        @blk.vector
        def _(eng):
            # et7 rides on DVE: after its 4 early-tile copies the DVE is
            # idle while Scalar serially evacuates 4,5,6; the last
            # (critical-path) tile overlaps Scalar's et6 work. Sync ships
            # it on the 5th s_cpv.
            ve_tiles = (0, 1, 2, 3, 7) if use_tail else (0, 1, 2, 3)
            for et in ve_tiles:
                eng.wait_ge(s_fin, FIN_THRESH[et])
                eng.tensor_copy(osb[:, et, :], ps[et][:]).then_inc(s_cpv, 1)

    nc.compile()
    return nc


_cache = {}


def _prep_inputs(x, G, W, b):
    x = np.asarray(x, dtype=np.float32)
    G = np.asarray(G, dtype=np.float32)
    W = np.asarray(W, dtype=np.float32)
    b = np.asarray(b, dtype=np.float32)

    X2 = np.ascontiguousarray(x.reshape(V, KD))                 # [V, (k,d)]
    WM = np.ascontiguousarray(W.transpose(2, 1, 3, 0).reshape(KD, CO))
    bias = b.sum(axis=-1).T.reshape(CO)                          # [(c,o)]
    Z2 = (X2 @ WM + bias[None, :]) * MU                          # [V, CO] scaled

    VS8 = N8 * 128  # 1792 fp8 contraction rows
    # fp8 part, partition-major [128, N8, CO]
    Z28P = np.ascontiguousarray(
        np.clip(Z2[:VS8], -240, 240)
        .astype(F8E4)
        .reshape(N8, 128, CO)
        .transpose(1, 0, 2)
    )
    Z2P = np.ascontiguousarray(
        Z2[VS8:].astype(BF16).reshape(NB, 128, CO).transpose(1, 0, 2)
    )

    GT = G.T * LAM                                               # [V, E] scaled
    GT8 = np.clip(GT[:VS8], -240, 240).astype(F8E4)
    GTB = GT[VS8:].astype(BF16)
    in_maps = []
    for c in range(N_CORES):
        sl = slice(c * EL, (c + 1) * EL)
        GT8P = np.ascontiguousarray(
            GT8[:, sl].reshape(N8, 128, EL).transpose(1, 0, 2)
        )
        GTP = np.ascontiguousarray(
            GTB[:, sl].reshape(NB, 128, EL).transpose(1, 0, 2)
        )
        in_maps.append({"gt8": GT8P, "z28": Z28P, "gt": GTP, "z2": Z2P})
    return in_maps


def _run(x, G, W, b, trace=False, trace_cores=None):
    from concourse.bass_utils import run_bass_kernel_spmd

    if "raw" not in _cache:
        _cache["raw"] = _build_bass_raw()
    nc = _cache["raw"]

    in_maps = _prep_inputs(x, G, W, b)
    kw = {}
    if trace_cores is not None:
        kw["trace_cores"] = trace_cores
    res = run_bass_kernel_spmd(
        nc, in_maps, core_ids=list(range(N_CORES)), trace=trace, **kw,
    )
    # out is [128, 8, 512] fp16 per core (scaled by LAM*MU), row
    # e_loc = et*128 + p
    outs = []
    for c in range(N_CORES):
        o = res.results[c]["out"]
        outs.append(np.ascontiguousarray(o.transpose(1, 0, 2)).reshape(EL, CO))
    out = np.concatenate(outs, axis=0).astype(np.float32) * (1.0 / (LAM * MU))
    out = out.reshape(E, C, O)
    return out, res


def kernel(x, G, W, b):
    out, _ = _run(x, G, W, b, trace=False)
    return out


# revision 21
# speedup vs baseline: 1.1549x; 1.0105x over previous
"""Trainium2 Bass kernel for nn_BlockLTN (gnn_message_passing).

Math:
    z[o,v,c] = sum_{k,d} x[v,k,d] * W[o,d,k,c] + sum_d b[o,c,d]
    out[e,c,o] = sum_v G[e,v] * z[o,v,c]

Folded:  out[e, c*8+o] = G[e,:] @ Z2[:, c*8+o]
  where  Z2[v, c*8+o] = (x.reshape(V,KD) @ W.transpose(2,1,3,0).reshape(KD,CO))[v, c*8+o]
                        + b.sum(-1).T.reshape(CO)[c*8+o]

The dominant work is the [E,V] @ [V, CO] GEMM over the 256 MB boundary
operator G (68.7 GFLOP); Z2 is a 4.3 GFLOP preprocessing folded on host.
Sharding (per hint): G and out row-wise over E across 8 cores (data
parallel over out-simplices); Z2 replicated; no collectives.

MIXED PRECISION: the first N8=14 v-chunks (1792 of 8192 contraction
rows, 21.9%) run as 7 fp8-e4m3 DoubleRow matmuls per e-tile (2 weights
per PE cell, 2 MACs/cycle); the remaining 50 chunks run bf16 at the
1 column/cycle silicon limit (216 ns per 128x128x512 MM). Error budget:
e4m3 quantization of both operands costs ~3.7% rel err at full
coverage; at 21.9% coverage the measured host-sim error is 1.76e-2 vs
the 2e-2 gate (bf16-only was 2.35e-3). Scaling: G ships x128 and Z2 x8
(exact powers of two, applied to BOTH precisions so the mixed PSUM
accumulation is uniform; G*128 lifts sigma to 2.56 so values clear the
e4m3 min-normal 0.015625 -- unscaled, 56% of G is subnormal garbage);
the host divides the fp16 output by 1024 (max |out*1024| ~ 27k < 65504).
fp8 pairs run FIRST: the HAM-cold window (~3.4 us at 1.2 GHz from
kernel start) covers work at 2 MACs/cycle, and the delivery-bound ramp
needs half the DMA bytes.

SYNCHRONIZATION (inherited, load-bearing): a DMA's `.then_inc(sem, 16)`
is performed as SIXTEEN independent +1 increments, one per SDMA engine.
With several DMAs in flight on one semaphore, a later DMA's increments
can satisfy an earlier DMA's cumulative threshold while one lagging
SDMA engine still owes its 8 partitions -> rank-8 stale-data corruption
(observed under NTFF profiling skew). Therefore every DMA a consumer
waits on gets its OWN single-use semaphore (threshold 16 == fully
landed).

Tail (v=63, bf16, unchanged order 0..7 -- a reordered tail shipped
garbage intermittently in a prior session): Vector evacuates e-tiles
0-3 and 7 (PSUM->SBUF fp16), Scalar 4-6; Sync ships 0:2, 2:4 and 7,
Scalar ships 4:6, 6. The final all-DMAs-landed wait (s_out) is DROPPED:
the runtime's end-of-NEFF epilogue (all-engine barrier + ~6.9 us of
per-engine semaphore clears + final barrier, appended by the runtime
after this program) runs before the host can observe completion, and
the last out-DMA receipt (~3.2 us) lands well inside it. s_out residue
from late receipts is wiped by the start-of-block clear on the next
execution.

Preamble: the profiler's exec-time clock starts at the framework's
const-tile MEMSETs (~6.05 us into the runtime preamble); the two DMAs
gating the first matmul (z28 chunks 0:2 and gt8 chunks 0:2 cols 0:512)
are relocated into the preamble right after those MEMSETs on the
Scalar queue -- past the runtime's ~2.5 us exec-start DMA-init race
window that corrupted a start-of-preamble variant, but ~1.2 us before
the all-engine barrier releases.
"""

import os

import numpy as np
import ml_dtypes

V = 8192
E = 8192
K = 64
C = 64
D = 8
O = 8
KD = K * D    # 512
CO = C * O    # 512
N_CORES = 8
EL = E // N_CORES  # 1024 out-rows per core
N_VCHUNK = V // 128  # 64
N_ETILE = EL // 128  # 8

N8 = 16               # fp8 chunks (v 0..15) -> 8 DoubleRow pairs
NP8 = N8 // 2
NB = N_VCHUNK - N8    # 48 bf16 chunks (v 16..63)
NSLOT = 16            # bf16 gt ring depth
LAM = 128.0           # G scale (2**7)
MU = 8.0              # Z2 scale (2**3)

BF16 = ml_dtypes.bfloat16
F8E4 = ml_dtypes.float8_e4m3  # TRN fp8_exp4: max +-240

# fp8 GT dma plan, in fp8-chunk spans. dma 0 (chunks 0:2, pre-issued
# mid-preamble on Scalar) ships WHOLE: a column split halves the
# per-partition contiguous run to 512 B and the descriptor overhead
# stretched the first flight 3.0 -> 4.3 us; unsplit it merges to 2 KB.
GT8_PLAN = [(0, 2), (2, 2), (4, 4), (8, 4), (12, 4)]
N_GT8_DMAS = len(GT8_PLAN)
Z28_GROUPS = [(0, 2), (2, 2), (4, 4), (8, 4), (12, 4)]  # first pre-issued

# bf16 GT dmas (chunks 16..63) through the 16-slot ring, slot (a-16)%16.
# Spans are chosen so no dma wraps the ring boundary.
GT_DMAS = [(16, 2), (18, 2)] + [(20 + 4 * t, 4) for t in range(11)]
assert sum(n for _, n in GT_DMAS) == NB
for _a, _n in GT_DMAS:
    assert (_a - N8) % NSLOT + _n <= NSLOT, (_a, _n)
_GT_IDX = {}
for _d, (_a, _n) in enumerate(GT_DMAS):
    for _c in range(_a, _a + _n):
        _GT_IDX[_c] = _d

Z2_GROUPS = [(16, 2), (18, 2)] + [(20 + 4 * t, 4) for t in range(11)]
assert sum(n for _, n in Z2_GROUPS) == NB

# v=63 e-tile emission order; each final MM bumps s_fin.
FIN_ORDER = list(range(N_ETILE))
FIN_THRESH = {et: et + 1 for et in FIN_ORDER}


def _build_bass_raw():
    import concourse.mybir as mybir
    from concourse import bacc

    f32 = mybir.dt.float32
    fp16 = mybir.dt.float16
    bf16 = mybir.dt.bfloat16
    f8e4 = mybir.dt.float8e4
    DR = mybir.MatmulPerfMode.DoubleRow

    nc = bacc.Bacc("TRN2", target_bir_lowering=False)

    # partition-major layouts prepared on host
    gt8 = nc.dram_tensor("gt8", (128, N8, EL), f8e4, kind="ExternalInput")
    z28 = nc.dram_tensor("z28", (128, N8, CO), f8e4, kind="ExternalInput")
    gt = nc.dram_tensor("gt", (128, NB, EL), bf16, kind="ExternalInput")
    z2 = nc.dram_tensor("z2", (128, NB, CO), bf16, kind="ExternalInput")
    out = nc.dram_tensor("out", (128, N_ETILE, CO), fp16, kind="ExternalOutput")

    gtsb8 = nc.alloc_sbuf_tensor("gtsb8", [128, N8, EL], f8e4)
    z2sb8 = nc.alloc_sbuf_tensor("z2sb8", [128, N8, CO], f8e4)
    gtsb = nc.alloc_sbuf_tensor("gtsb", [128, NSLOT, EL], bf16)
    z2sb = nc.alloc_sbuf_tensor("z2sb", [128, NB, CO], bf16)
    osb = nc.alloc_sbuf_tensor("osb", [128, N_ETILE, CO], fp16)
    ps = [nc.alloc_psum_tensor(f"ps{i}", [128, CO], f32) for i in range(N_ETILE)]

    # single-use DMA-completion sems (see module docstring)
    g8s = [nc.alloc_semaphore(f"s_g8{d}") for d in range(N_GT8_DMAS)]
    z28s = [nc.alloc_semaphore(f"s_z28{g}") for g in range(len(Z28_GROUPS))]
    gts = [nc.alloc_semaphore(f"s_gt{d}") for d in range(len(GT_DMAS))]
    z2s = [nc.alloc_semaphore(f"s_z2{g}") for g in range(len(Z2_GROUPS))]
    s_mm = nc.alloc_semaphore("s_mm")    # chunks consumed by the PE
    s_fin = nc.alloc_semaphore("s_fin")  # v=63 matmuls retired, FIN_ORDER
    s_cpv = nc.alloc_semaphore("s_cpv")  # DVE psum->sbuf copies done
    s_out = nc.alloc_semaphore("s_out")  # out DMAs landed: 16 each

    all_sems = g8s + z28s + gts + z2s + [s_mm, s_fin, s_cpv, s_out]
    nums = [s.num for s in all_sems]
    assert nums == list(range(nums[0], nums[0] + len(nums))), nums
    sem_range = range(nums[0], nums[-1] + 1)
    # cleared at start: only sems without pre-barrier producers (the
    # gt/z2 sems rely on NEFF-load zeroing + the end-of-kernel clear)
    sem_range_start = range(s_mm.num, s_out.num + 1)

    def z28_dma(eng, g):
        a, n = Z28_GROUPS[g]
        eng.dma_start(z2sb8[:, a:a + n, :], z28[:, a:a + n, :]).then_inc(
            z28s[g], 16
        )

    use_midpre = os.environ.get("KOPT_MIDPRE", "1") != "0"
    use_tail = os.environ.get("KOPT_TAIL", "1") != "0"
    use_nowait = os.environ.get("KOPT_NOWAIT", "1") != "0"

    if use_midpre:
        # Relocate the two first-matmul-gating DMAs to just after the
        # framework's const-tile memsets (~5.9 us in, past the runtime's
        # ~2.5 us exec-start DMA-init race window, before the all-engine
        # barrier). gt8 first: the PE's LDWEIGHTS needs it before the
        # rhs, so its flight should land first.
        entry = nc.main_func.blocks[0]
        pre_n = len(entry.instructions)
        nc.scalar.dma_start(gtsb8[:, 0:2, :], gt8[:, 0:2, :]).then_inc(
            g8s[0], 16
        )
        z28_dma(nc.scalar, 0)
        mine = entry.instructions[pre_n:]
        assert len(mine) == 2, len(mine)
        del entry.instructions[pre_n:]
        anchor = 1 + max(
            i for i, ins in enumerate(entry.instructions)
            if type(ins).__name__ == "InstMemset"
        )
        for off, ins in enumerate(mine):
            entry.instructions.insert(anchor + off, ins)

    with nc.Block(name="k", no_gpsimd_drain=True) as blk:

        @blk.sync
        def _(eng):
            eng.sem_clear(sem_range_start)
            if not use_midpre:
                eng.dma_start(gtsb8[:, 0:2, :], gt8[:, 0:2, :]).then_inc(
                    g8s[0], 16
                )
            for i, (a, n) in enumerate(GT8_PLAN[1:]):
                # fp8 spans ship unpaced: the whole fp8 working set is
                # needed within the first ~24 us and ramp-window DMA
                # latency is 4-7 us under 8-core congestion
                eng.dma_start(
                    gtsb8[:, a:a + n, :], gt8[:, a:a + n, :]
                ).then_inc(g8s[1 + i], 16)
            # The HWDGE queue is FIFO and per-queue bandwidth-limited
            # (~160 GB/s observed under 8-core load); splitting the 12 MB
            # bf16 GT stream across BOTH queues keeps each under the cap.
            # Even-indexed spans ship here, odd-indexed on Scalar.
            for d, (a, n) in enumerate(GT_DMAS):
                if d % 2 == 1:
                    continue
                # paced ~8 chunks ahead of consumption; the max() term
                # covers ring-slot reuse (chunk a lands in the slot chunk
                # a-16 occupied)
                eng.wait_ge(s_mm, max(a + n - NSLOT, a - 8))
                sl = (a - N8) % NSLOT
                eng.dma_start(
                    gtsb[:, sl:sl + n, :], gt[:, a - N8:a - N8 + n, :]
                ).then_inc(gts[d], 16)
            for k, et in enumerate((0, 2)):
                eng.wait_ge(s_cpv, 2 * (k + 1))
                eng.dma_start(
                    out[:, et:et + 2, :], osb[:, et:et + 2, :]
                ).then_inc(s_out, 16)
            if use_tail:
                # e-tile 7 evacuated by DVE (5th s_cpv inc), shipped here
                eng.wait_ge(s_cpv, 5)
                eng.dma_start(
                    out[:, 7:8, :], osb[:, 7:8, :]
                ).then_inc(s_out, 16)
            if not use_nowait:
                eng.wait_ge(s_out, 16 * 5)
            # leave sems zeroed so a re-execution of the loaded NEFF works
            eng.sem_clear(sem_range)

        @blk.scalar
        def _(eng):
            for g in range(0 if not use_midpre else 1, len(Z28_GROUPS)):
                z28_dma(eng, g)
            # merged z2 + odd-indexed gt spans, in consumption order (see
            # the queue-balance note on the sync engine)
            merged = [("z2", g, c0, n) for g, (c0, n) in enumerate(Z2_GROUPS)]
            merged += [
                ("gt", d, a, n)
                for d, (a, n) in enumerate(GT_DMAS)
                if d % 2 == 1
            ]
            merged.sort(key=lambda r: (r[2], r[0] != "z2"))
            for kind, idx, c0, n in merged:
                if kind == "z2":
                    eng.wait_ge(s_mm, c0 - 8)
                    eng.dma_start(
                        z2sb[:, c0 - N8:c0 - N8 + n, :],
                        z2[:, c0 - N8:c0 - N8 + n, :],
                    ).then_inc(z2s[idx], 16)
                else:
                    eng.wait_ge(s_mm, max(c0 + n - NSLOT, c0 - 8))
                    sl = (c0 - N8) % NSLOT
                    eng.dma_start(
                        gtsb[:, sl:sl + n, :], gt[:, c0 - N8:c0 - N8 + n, :]
                    ).then_inc(gts[idx], 16)
            sc_tiles = (4, 5, 6) if use_tail else (4, 5, 6, 7)
            for et in sc_tiles:
                eng.wait_ge(s_fin, FIN_THRESH[et])
                eng.copy(osb[:, et, :], ps[et][:])
                if et == 5:
                    eng.dma_start(
                        out[:, 4:6, :], osb[:, 4:6, :]
                    ).then_inc(s_out, 16)
                elif et >= 6:
                    eng.dma_start(
                        out[:, et:et + 1, :], osb[:, et:et + 1, :]
                    ).then_inc(s_out, 16)

        @blk.tensor
        def _(eng):
            # HAM warm-up: the PE exits the preamble barrier ~2.1 us before
            # the first input chunks land; garbage matmuls (into ps[0],
            # reset by the real start=True) keep the PE-busy activity
            # window saturated from barrier exit so the HAM un-throttle
            # (K=4/8 -> 8/8) fires ~1-2 us earlier into the real stream.
            # Data raced with the in-flight first DMAs is discarded.
            n_warm = int(os.environ.get("KOPT_WARM", "4"))
            for i in range(n_warm):
                eng.matmul(
                    ps[0][:],
                    lhsT=gtsb8[:, 0:2, 0:128],
                    rhs=z2sb8[:, 0:2, :],
                    start=(i == 0),
                    stop=(i == n_warm - 1),
                    perf_mode=DR,
                )
            # Build per-step (fp8 pair / bf16 chunk) wait lists, then emit
            # each step's waits just before the PREVIOUS step's last MM:
            # at a boundary the in-order PE queue otherwise serializes
            # [wait][LDWEIGHTS][MM], exposing the ~110-210 ns LDWEIGHTS
            # that mid-chunk hides behind the running MM (observed as
            # 432 ns boundary gaps).
            steps = []
            landed8 = 0
            g8 = 0
            for t in range(NP8):
                w = []
                while landed8 < 2 * t + 2:
                    w.append(z28s[g8])
                    landed8 += Z28_GROUPS[g8][1]
                    g8 += 1
                for i, (a, n) in enumerate(GT8_PLAN):
                    if 2 * t == a:
                        w.append(g8s[i])
                steps.append(("f8", t, w))
            landed = N8
            g = 0
            for v in range(N8, N_VCHUNK):
                w = []
                while v >= landed:
                    w.append(z2s[g])
                    landed += Z2_GROUPS[g][1]
                    g += 1
                d = _GT_IDX[v]
                if v == GT_DMAS[d][0]:
                    w.append(gts[d])
                steps.append(("bf", v, w))
            for si, (kind, idx, waits) in enumerate(steps):
                if si == 0:
                    for s in waits:
                        eng.wait_ge(s, 16)
                nxt = steps[si + 1][2] if si + 1 < len(steps) else []
                for j in range(N_ETILE):
                    et = j
                    if j == N_ETILE - 1:
                        for s in nxt:
                            eng.wait_ge(s, 16)
                    if kind == "f8":
                        t = idx
                        mm = eng.matmul(
                            ps[et][:],
                            lhsT=gtsb8[
                                :, 2 * t:2 * t + 2, et * 128:(et + 1) * 128
                            ],
                            rhs=z2sb8[:, 2 * t:2 * t + 2, :],
                            start=(t == 0),
                            stop=False,
                            perf_mode=DR,
                        )
                        if j == N_ETILE - 1:
                            mm.then_inc(s_mm, 2)
                    else:
                        v = idx
                        last = v == N_VCHUNK - 1
                        sl = (v - N8) % NSLOT
                        mm = eng.matmul(
                            ps[et][:],
                            lhsT=gtsb[:, sl, et * 128:(et + 1) * 128],
                            rhs=z2sb[:, v - N8, :],
                            start=False,
                            stop=last,
                        )
                        if j == N_ETILE - 1 and not last:
                            mm.then_inc(s_mm, 1)
                        if last:
                            mm.then_inc(s_fin, 1)

        @blk.vector
        def _(eng):
            # et7 rides on DVE: after its 4 early-tile copies the DVE is
            # idle while Scalar serially evacuates 4,5,6; the last
            # (critical-path) tile overlaps Scalar's et6 work. Sync ships
            # it on the 5th s_cpv.
            ve_tiles = (0, 1, 2, 3, 7) if use_tail else (0, 1, 2, 3)
            for et in ve_tiles:
                eng.wait_ge(s_fin, FIN_THRESH[et])
                eng.tensor_copy(osb[:, et, :], ps[et][:]).then_inc(s_cpv, 1)

    nc.compile()
    return nc


_cache = {}


def _prep_inputs(x, G, W, b):
    x = np.asarray(x, dtype=np.float32)
    G = np.asarray(G, dtype=np.float32)
    W = np.asarray(W, dtype=np.float32)
    b = np.asarray(b, dtype=np.float32)

    X2 = np.ascontiguousarray(x.reshape(V, KD))                 # [V, (k,d)]
    WM = np.ascontiguousarray(W.transpose(2, 1, 3, 0).reshape(KD, CO))
    bias = b.sum(axis=-1).T.reshape(CO)                          # [(c,o)]
    Z2 = (X2 @ WM + bias[None, :]) * MU                          # [V, CO] scaled

    VS8 = N8 * 128  # 1792 fp8 contraction rows
    # fp8 part, partition-major [128, N8, CO]
    Z28P = np.ascontiguousarray(
        np.clip(Z2[:VS8], -240, 240)
        .astype(F8E4)
        .reshape(N8, 128, CO)
        .transpose(1, 0, 2)
    )
    Z2P = np.ascontiguousarray(
        Z2[VS8:].astype(BF16).reshape(NB, 128, CO).transpose(1, 0, 2)
    )

    GT = G.T * LAM                                               # [V, E] scaled
    GT8 = np.clip(GT[:VS8], -240, 240).astype(F8E4)
    GTB = GT[VS8:].astype(BF16)
    in_maps = []
    for c in range(N_CORES):
        sl = slice(c * EL, (c + 1) * EL)
        GT8P = np.ascontiguousarray(
            GT8[:, sl].reshape(N8, 128, EL).transpose(1, 0, 2)
        )
        GTP = np.ascontiguousarray(
            GTB[:, sl].reshape(NB, 128, EL).transpose(1, 0, 2)
        )
        in_maps.append({"gt8": GT8P, "z28": Z28P, "gt": GTP, "z2": Z2P})
    return in_maps


def _run(x, G, W, b, trace=False, trace_cores=None):
    from concourse.bass_utils import run_bass_kernel_spmd

    if "raw" not in _cache:
        _cache["raw"] = _build_bass_raw()
    nc = _cache["raw"]

    in_maps = _prep_inputs(x, G, W, b)
    kw = {}
    if trace_cores is not None:
        kw["trace_cores"] = trace_cores
    res = run_bass_kernel_spmd(
        nc, in_maps, core_ids=list(range(N_CORES)), trace=trace, **kw,
    )
    # out is [128, 8, 512] fp16 per core (scaled by LAM*MU), row
    # e_loc = et*128 + p
    outs = []
    for c in range(N_CORES):
        o = res.results[c]["out"]
        outs.append(np.ascontiguousarray(o.transpose(1, 0, 2)).reshape(EL, CO))
    out = np.concatenate(outs, axis=0).astype(np.float32) * (1.0 / (LAM * MU))
    out = out.reshape(E, C, O)
    return out, res


def kernel(x, G, W, b):
    out, _ = _run(x, G, W, b, trace=False)
    return out
